# revision 15
# baseline (speedup 1.0000x reference)
"""ProteinInterfacePrediction fused Bass kernel for 8 TRN2 NeuronCores.

Sharding: core c = (batch b = c//2, L-half h = c%2); each core computes its
(256, 512) output tile.

Decomposition (validated bit-level in numpy vs the jax reference):
  - GNN residual folded into HOPI: pl = Wl@nodeT + (Wl/16)@S, S = sum_k tanh(hn+he)
  - conv1 is rank-separable before relu: conv1(P) = U[co,l] + V[co,r] (+consts),
    boundary columns via mask-augmented 1-D convs, boundary rows via per-core
    flag-baked V-weight variants.
  - conv2 on TensorE: 4-input-row blocks on 128 partitions (K = 4rows x 32ci),
    stride-2 (P/Q dual layouts), 3 dr-taps, 4-way 32-column array tiling.
  - conv3 (1x1) + bias + sigmoid fused at the tail.

Dispatch layer (dominates wall-clock over the axon tunnel):
  - ALL wire data packed into ONE uint8 array per core (regions bitcast to
    bf16 / fp8e4m3 / f32 on device); edge features shipped fp8, nodes and
    weights bf16
  - pair-shared data (weights + receptor features, identical for the two
    cores of a batch) uploaded once per pair as half-blobs and AllGather'd
    on-device over NeuronLink
  - donated output buffers created on-device (no zero upload), prefetched
    for the next call
  - output sigmoid quantized to uint8 (1/255 steps) and AllGather'd
    on-device, so the host fetches a single 1 MB shard in one round trip
  - the jitted SPMD callable is built once and cached (no per-call
    recompile)
"""

import numpy as np
import ml_dtypes

B, L, R, KNB = 4, 512, 512, 16
DN, DE = 128, 64
NLIG = 260           # 256 + 2-row halo each side
NREC = 512
PLIG = NLIG * KNB
PREC = NREC * KNB
CH = 64              # gnn nodes per chunk
NSTRIP = 8

BF16 = ml_dtypes.bfloat16

# (name, rows, cols) — single source of truth for host + device.
OCT_LAYOUT = [       # identical on all 8 cores (weights)
    ("WNT", 128, 128), ("WETb", 64, 128),
    ("WlT", 128, 32), ("WlT16", 128, 32), ("WrT", 128, 32), ("WrT16", 128, 32),
    ("UW", 32, 96), ("A0W", 32, 96), ("A511W", 32, 96),
    ("W1c0", 32, 192), ("W1c511", 32, 192),
    ("c0const", 1, 96), ("c511const", 1, 96),
    ("VW", 32, 384), ("VC", 1, 128),
    ("W2P0", 128, 96), ("W2P1", 128, 96), ("W3selb", 128, 4),
    ("ONE1", 1, 1), ("ONESR", 1, 512),
]
PAIR_LAYOUT = [      # shared between the two cores of a batch
    ("recnodeT", 128, NREC),
]
CORE_LAYOUT = [      # genuinely per-core (h-dependent or ligand slice)
    ("VWfirst", 32, 384), ("VWqlast", 32, 384),
    ("VCfirst", 1, 128), ("VCqlast", 1, 128),
    ("plmask", 32, 260), ("plmaskrow", 1, 260),
    ("lignodeT", 128, NLIG),
]
PAIRF8_LAYOUT = [      # fp8 pair-shared (receptor edges)
    ("recedgeT", 64, PREC),
]
COREF8_LAYOUT = [      # fp8 per-core (ligand edge slice)
    ("ligedgeT", 64, PLIG),
]
F32_LAYOUT = [
    ("gnnbias", 128, 1), ("bc2rep", 128, 1), ("b3vec", 128, 1),
    ("rmP0", 128, 1), ("rmQ63", 128, 1),
]


def _offsets(layout):
    offs, off = {}, 0
    for name, r, c in layout:
        offs[name] = (off, r, c)
        off += r * c
    return offs, off


OCT_OFFS, _OT = _offsets(OCT_LAYOUT)            # bf16 elements
PAIR_OFFS, _PT = _offsets(PAIR_LAYOUT)          # bf16 elements
CORE_OFFS, CORE_TOTAL = _offsets(CORE_LAYOUT)   # bf16 elements
PAIRF8_OFFS, PAIRF8_TOTAL = _offsets(PAIRF8_LAYOUT)   # fp8 elements
COREF8_OFFS, COREF8_TOTAL = _offsets(COREF8_LAYOUT)   # fp8 elements
F32_OFFS, F32_TOTAL = _offsets(F32_LAYOUT)      # f32 elements
FP8 = ml_dtypes.float8_e4m3

def _pad4(n):
    return (n + 3) & ~3

# byte layout of the oct (8-way shared) weight blob, 1/8 uploaded per core
OCT_BYTES = 2 * _OT
OCT_BYTES += (-OCT_BYTES) % 32768    # eighths stay 4096-aligned for 2-D DMA
OCT_PC = OCT_BYTES // 8
# byte layout of the pair blob: [bf16 region][fp8 region], halved for upload
PAIR_BF_BYTES = _pad4(2 * _PT)
PAIR_BYTES = PAIR_BF_BYTES + _pad4(PAIRF8_TOTAL)
PAIR_BYTES += (-PAIR_BYTES) % 8192   # halves stay 4096-aligned for 2-D DMA
PH_BYTES = PAIR_BYTES // 2
# byte layout of the master wire pack:
#   [oct eighth][pair half][core bf16][core fp8][f32]
PAIR_OFF = OCT_PC
CORE_BF_OFF = PAIR_OFF + PH_BYTES
CORE_F8_OFF = CORE_BF_OFF + _pad4(2 * CORE_TOTAL)
F32_OFF = CORE_F8_OFF + _pad4(COREF8_TOTAL)
NBYTES = F32_OFF + 4 * F32_TOTAL

_CACHE = {}


def _host_prep(inputs):
    """Pack inputs into (8, PAIR_HALF) + (8, CORE_TOTAL) bf16 + (8, F32_TOTAL) f32."""
    f32 = np.float32
    W1 = np.asarray(inputs['Wc1'], f32)
    W2 = np.asarray(inputs['Wc2'], f32)
    W3 = np.asarray(inputs['Wc3'], f32)[0, :, 0, 0]
    b1 = np.asarray(inputs['bc1'], f32)
    b2 = np.asarray(inputs['bc2'], f32)
    b3 = float(np.asarray(inputs['bc3'], f32)[0])
    Wp = np.asarray(inputs['Wp'], f32)
    bp = np.asarray(inputs['bp'], f32)
    Wl, Wr = Wp[:, :DN], Wp[:, DN:]
    WN = np.asarray(inputs['WN'], f32)
    bN = np.asarray(inputs['bN'], f32)
    WE = np.asarray(inputs['WE'], f32)
    bE = np.asarray(inputs['bE'], f32)

    A = W1.sum(axis=3)
    Wv = W1.sum(axis=2)
    cU = np.einsum('oidr,i->od', W1, bp)

    sh = {}
    sh['WNT'] = WN.T
    sh['WETb'] = WE.T
    sh['WlT'] = Wl.T
    sh['WlT16'] = (Wl / 16.0).T
    sh['WrT'] = Wr.T
    sh['WrT16'] = (Wr / 16.0).T

    def pack3(M):  # (co, ci, dl) -> [32, 96] of [ci, co] blocks
        out = np.zeros((32, 96), f32)
        for dl in range(3):
            out[:, 32 * dl:32 * dl + 32] = M[:, :, dl].T
        return out

    sh['UW'] = pack3(A)
    sh['A0W'] = pack3(W1[:, :, :, 1:].sum(axis=3))
    sh['A511W'] = pack3(W1[:, :, :, :2].sum(axis=3))

    W1c0 = np.zeros((32, 192), f32)
    W1c511 = np.zeros((32, 192), f32)
    for dl in range(3):
        for t, dr in enumerate((1, 2)):
            W1c0[:, 32 * (2 * dl + t):32 * (2 * dl + t) + 32] = W1[:, :, dl, dr].T
        for t, dr in enumerate((0, 1)):
            W1c511[:, 32 * (2 * dl + t):32 * (2 * dl + t) + 32] = W1[:, :, dl, dr].T
    sh['W1c0'], sh['W1c511'] = W1c0, W1c511

    c0c = np.zeros((1, 96), f32)
    c511c = np.zeros((1, 96), f32)
    for dl in range(3):
        c0c[0, 32 * dl:32 * dl + 32] = np.einsum('oid,i->o', W1[:, :, dl, 1:], bp)
        c511c[0, 32 * dl:32 * dl + 32] = np.einsum('oid,i->o', W1[:, :, dl, :2], bp)
    c0c[0, 32:64] += b1
    c511c[0, 32:64] += b1
    sh['c0const'], sh['c511const'] = c0c, c511c

    VW = np.zeros((32, 384), f32)
    for dr in range(3):
        blk = Wv[:, :, dr].T
        for j in range(4):
            VW[:, 128 * dr + 32 * j:128 * dr + 32 * j + 32] = blk
    sh['VW'] = VW
    vc = cU.sum(axis=1) + b1
    VC = np.tile(vc, 4).reshape(1, 128)
    sh['VC'] = VC

    W2P0 = np.zeros((128, 96), f32)
    W2P1 = np.zeros((128, 96), f32)
    for dr in range(3):
        for j in range(3):
            W2P0[32 * j:32 * j + 32, 32 * dr:32 * dr + 32] = W2[:, :, j, dr].T
        for j in range(1, 4):
            W2P1[32 * j:32 * j + 32, 32 * dr:32 * dr + 32] = W2[:, :, j - 1, dr].T
    sh['W2P0'], sh['W2P1'] = W2P0, W2P1

    W3sel = np.zeros((128, 4), f32)
    for j in range(4):
        W3sel[32 * j:32 * j + 32, j] = W3
    sh['W3selb'] = W3sel
    sh['ONE1'] = np.ones((1, 1), f32)
    sh['ONESR'] = np.ones((1, 512), f32)

    shf = {}
    shf['gnnbias'] = (bN + bE).reshape(DN, 1)
    shf['bc2rep'] = np.tile(b2, 4).reshape(128, 1)
    shf['b3vec'] = np.full((128, 1), b3, f32)

    # big features: convert to bf16 once, per batch, then slice per core
    lig_nf = np.asarray(inputs['ligand_node_features'], f32).astype(BF16)
    lig_ef = np.asarray(inputs['ligand_edge_features'], f32).astype(FP8)
    rec_nf = np.asarray(inputs['receptor_node_features'], f32).astype(BF16)
    rec_ef = np.asarray(inputs['receptor_edge_features'], f32).astype(FP8)
    lig_nfT = [np.ascontiguousarray(lig_nf[b].T) for b in range(B)]      # (128, L)
    lig_efT = [np.ascontiguousarray(lig_ef[b].reshape(L * KNB, DE).T)
               for b in range(B)]                                        # (64, L*K)
    rec_nfT = [np.ascontiguousarray(rec_nf[b].T) for b in range(B)]
    rec_efT = [np.ascontiguousarray(rec_ef[b].reshape(R * KNB, DE).T)
               for b in range(B)]

    master = np.zeros((8, NBYTES), np.uint8)
    packb = np.zeros((8, CORE_TOTAL), BF16)
    packb8 = np.zeros((8, COREF8_TOTAL), FP8)
    packf = np.zeros((8, F32_TOTAL), np.float32)

    octby = np.zeros(OCT_BYTES, np.uint8)
    octbf = octby[:2 * _OT].view(BF16)
    for name in ('WNT', 'WETb', 'WlT', 'WlT16', 'WrT', 'WrT16', 'UW',
                 'A0W', 'A511W', 'W1c0', 'W1c511', 'c0const', 'c511const',
                 'VW', 'VC', 'W2P0', 'W2P1', 'W3selb', 'ONE1', 'ONESR'):
        off, r, c = OCT_OFFS[name]
        a = np.asarray(sh[name])
        assert a.shape == (r, c), (name, a.shape, (r, c))
        octbf[off:off + r * c] = a.astype(BF16, copy=False).ravel()
    for core in range(8):
        master[core, :OCT_PC] = octby[core * OCT_PC:(core + 1) * OCT_PC]

    for b in range(B):
        blobby = np.zeros(PAIR_BYTES, np.uint8)
        blob = blobby[:2 * _PT].view(BF16)
        off, r, c = PAIR_OFFS['recnodeT']
        blob[off:off + r * c] = rec_nfT[b].ravel()
        o8, r8, c8 = PAIRF8_OFFS['recedgeT']
        blobby[PAIR_BF_BYTES + o8:PAIR_BF_BYTES + o8 + r8 * c8] = \
            rec_efT[b].reshape(-1).view(np.uint8)
        master[2 * b, PAIR_OFF:PAIR_OFF + PH_BYTES] = blobby[:PH_BYTES]
        master[2 * b + 1, PAIR_OFF:PAIR_OFF + PH_BYTES] = blobby[PH_BYTES:]

    for core in range(8):
        b, h = core // 2, core % 2
        lo = 256 * h - 2

        def putb(name, arr):
            off, r, c = CORE_OFFS[name]
            a = np.asarray(arr)
            assert a.shape == (r, c), (name, a.shape, (r, c))
            packb[core, off:off + r * c] = a.astype(BF16, copy=False).ravel()

        def putf(name, arr):
            off, r, c = F32_OFFS[name]
            packf[core, off:off + r * c] = np.asarray(arr, np.float32).ravel()

        for name in ('gnnbias', 'bc2rep', 'b3vec'):
            putf(name, shf[name])

        lig_node = np.zeros((128, NLIG), BF16)
        lig_edge = np.zeros((64, PLIG), FP8)
        g0, g1 = max(lo, 0), min(lo + 260, L)
        lig_node[:, g0 - lo:g1 - lo] = lig_nfT[b][:, g0:g1]
        lig_edge[:, (g0 - lo) * KNB:(g1 - lo) * KNB] = \
            lig_efT[b][:, g0 * KNB:g1 * KNB]
        putb('lignodeT', lig_node)
        o8, r8, c8 = COREF8_OFFS['ligedgeT']
        packb8[core, o8:o8 + r8 * c8] = lig_edge.ravel()

        plmask = np.array([1.0 if 0 <= lo + i < L else 0.0 for i in range(260)],
                          np.float32)
        putb('plmask', np.tile(plmask.reshape(1, 260), (32, 1)))
        putb('plmaskrow', plmask.reshape(1, 260))

        flag0 = 1.0 if h == 0 else 0.0
        flag1 = 1.0 if h == 1 else 0.0
        VWfirst = sh['VW'].copy()
        VWqlast = sh['VW'].copy()
        for dr in range(3):
            VWfirst[:, 128 * dr + 32:128 * dr + 64] -= flag0 * W1[:, :, 0, dr].T
            VWqlast[:, 128 * dr + 64:128 * dr + 96] -= flag1 * W1[:, :, 2, dr].T
        putb('VWfirst', VWfirst)
        putb('VWqlast', VWqlast)

        VCfirst, VCqlast = VC.copy(), VC.copy()
        VCfirst[0, 32:64] -= flag0 * cU[:, 0]
        VCqlast[0, 64:96] -= flag1 * cU[:, 2]
        putb('VCfirst', VCfirst)
        putb('VCqlast', VCqlast)

        rmP0 = np.ones((128, 1), f32)
        rmQ63 = np.ones((128, 1), f32)
        for j in range(4):
            if not (0 <= 256 * h + (j - 1) < L):
                rmP0[32 * j:32 * j + 32] = 0.0
            if not (0 <= 256 * h + (253 + j) < L):
                rmQ63[32 * j:32 * j + 32] = 0.0
        putf('rmP0', rmP0)
        putf('rmQ63', rmQ63)
        master[core, CORE_BF_OFF:CORE_BF_OFF + 2 * CORE_TOTAL] = \
            packb[core].view(np.uint8)
        master[core, CORE_F8_OFF:CORE_F8_OFF + COREF8_TOTAL] = \
            packb8[core].view(np.uint8)
        master[core, F32_OFF:F32_OFF + 4 * F32_TOTAL] = \
            packf[core].view(np.uint8)
    return (master,)


def _build_program():
    import concourse.bacc as bacc
    import concourse.mybir as mybir
    from concourse.tile import TileContext

    dt = mybir.dt
    f32, bf16 = dt.float32, dt.bfloat16
    AF = mybir.ActivationFunctionType
    ALU = mybir.AluOpType

    nc = bacc.Bacc("TRN2", target_bir_lowering=False, debug=False, num_devices=8)

    f8 = dt.float8e4
    u8 = dt.uint8
    masterd = nc.dram_tensor("master", [1, NBYTES], u8, kind="ExternalInput")
    out = nc.dram_tensor("out", [8 * 512, 256], u8, kind="ExternalOutput")

    with TileContext(nc) as tc:
        with tc.tile_pool(name="const", bufs=1) as cpool, \
             tc.tile_pool(name="dstage", bufs=1, space="DRAM") as dpool:

            # ---- oct AllGather: weight eighth -> full weight blob ----
            obounce = dpool.tile([1, OCT_PC], u8)
            goct = dpool.tile([1, OCT_BYTES], u8)
            nc.gpsimd.dma_start(
                out=obounce[:].rearrange("o (r c) -> (o r) c", c=4096),
                in_=masterd[0:1, 0:OCT_PC].rearrange(
                    "o (r c) -> (o r) c", c=4096))
            nc.gpsimd.collective_compute(
                "AllGather", mybir.AluOpType.bypass,
                replica_groups=[[0, 1, 2, 3, 4, 5, 6, 7]],
                ins=[obounce[:]],
                outs=[goct[:].rearrange("o (a b) -> (o a) b", a=8)])

            # ---- pair AllGather: half-blob -> full shared blob (bytes) ----
            pbounce = dpool.tile([1, PH_BYTES], u8)
            gpair = dpool.tile([1, PAIR_BYTES], u8)
            nc.gpsimd.dma_start(
                out=pbounce[:].rearrange("o (r c) -> (o r) c", c=4096),
                in_=masterd[0:1, PAIR_OFF:PAIR_OFF + PH_BYTES].rearrange(
                    "o (r c) -> (o r) c", c=4096))
            nc.gpsimd.collective_compute(
                "AllGather", mybir.AluOpType.bypass,
                replica_groups=[[0, 1], [2, 3], [4, 5], [6, 7]],
                ins=[pbounce[:]],
                outs=[gpair[:].rearrange("o (a b) -> (o a) b", a=2)])

            def osrc(name):
                off, r, c = OCT_OFFS[name]
                return goct[0:1, 2 * off:2 * (off + r * c)].bitcast(
                    bf16).rearrange("o (r c) -> (o r) c", c=c)

            def gsrc(name):
                off, r, c = PAIR_OFFS[name]
                return gpair[0:1, 2 * off:2 * (off + r * c)].bitcast(
                    bf16).rearrange("o (r c) -> (o r) c", c=c)

            def bsrc(name):
                off, r, c = CORE_OFFS[name]
                bo = CORE_BF_OFF + 2 * off
                return masterd[0:1, bo:bo + 2 * r * c].bitcast(
                    bf16).rearrange("o (r c) -> (o r) c", c=c)

            def fsrc(name):
                off, r, c = F32_OFFS[name]
                bo = F32_OFF + 4 * off
                return masterd[0:1, bo:bo + 4 * r * c].bitcast(
                    f32).rearrange("o (r c) -> (o r) c", c=c)

            def ctile(name, dtype=bf16, src=None, offs=None):
                off, r, c = offs[name]
                t = cpool.tile([128, c], dtype, tag=f"c_{name}")
                nc.sync.dma_start(out=t[0:r, 0:c], in_=src(name))
                return t

            def ptile(name):
                return ctile(name, src=osrc, offs=OCT_OFFS)

            def btile(name):
                return ctile(name, src=bsrc, offs=CORE_OFFS)

            def ftile(name):
                return ctile(name, dtype=f32, src=fsrc, offs=F32_OFFS)

            WNT_s = ptile("WNT")
            WETb_s = cpool.tile([128, 128], f8, tag="c_WETb")
            nc.gpsimd.dma_start(out=WETb_s[0:64, 0:128], in_=osrc("WETb"))
            gnnbias_s = ftile("gnnbias")
            WlT_s, WlT16_s = ptile("WlT"), ptile("WlT16")
            WrT_s, WrT16_s = ptile("WrT"), ptile("WrT16")
            UW_s = ptile("UW")
            W1c0_s, W1c511_s = ptile("W1c0"), ptile("W1c511")
            c0c_s, c511c_s = ptile("c0const"), ptile("c511const")
            VW_s, VWf_s, VWq_s = ptile("VW"), btile("VWfirst"), btile("VWqlast")
            VC_s, VCf_s, VCq_s = ptile("VC"), btile("VCfirst"), btile("VCqlast")
            W2P0_s, W2P1_s = ptile("W2P0"), ptile("W2P1")
            W3sel_s = ptile("W3selb")
            bc2rep_s, b3vec_s = ftile("bc2rep"), ftile("b3vec")
            ONE1_s, ONESR_s = ptile("ONE1"), ptile("ONESR")
            plmask_s = btile("plmask")
            plmaskrow_s = btile("plmaskrow")
            rmP0_s, rmQ63_s = ftile("rmP0"), ftile("rmQ63")
            nodeT_lig = btile("lignodeT")
            nodeT_rec = ctile("recnodeT", src=gsrc, offs=PAIR_OFFS)

            lo8 = CORE_F8_OFF + COREF8_OFFS["ligedgeT"][0]
            ligedge_flat = masterd[0:1, lo8:lo8 + 64 * PLIG].bitcast(
                f8).rearrange("o (r c) -> (o r) c", c=PLIG)
            ro8 = PAIR_BF_BYTES + PAIRF8_OFFS["recedgeT"][0]
            recedge_flat = gpair[0:1, ro8:ro8 + 64 * PREC].bitcast(
                f8).rearrange("o (r c) -> (o r) c", c=PREC)

            S_lig = cpool.tile([128, NLIG], bf16)
            S_rec = cpool.tile([128, NREC], bf16)
            plzA = cpool.tile([128, 260], bf16)     # rows 0-31 plz, row 32 mask
            przA = cpool.tile([128, 514], bf16)
            U_sb = cpool.tile([128, 260], f32)
            Uc0_sb = cpool.tile([128, 260], f32)
            Uc511_sb = cpool.tile([128, 260], f32)
            A0AUG = cpool.tile([128, 96], bf16)
            A511AUG = cpool.tile([128, 96], bf16)
            V_rep = cpool.tile([128, 512], f32)
            V_first = cpool.tile([128, 512], f32)
            V_qlast = cpool.tile([128, 512], f32)
            uP = cpool.tile([128, 64], f32, tag="uP")
            uQ = cpool.tile([128, 64], f32, tag="uQ")
            uc0P = cpool.tile([128, 64], f32, tag="uc0P")
            uc0Q = cpool.tile([128, 64], f32, tag="uc0Q")
            uc511P = cpool.tile([128, 64], f32, tag="uc511P")
            uc511Q = cpool.tile([128, 64], f32, tag="uc511Q")

            stage = dpool.tile([512, 256], u8)
            gout = dpool.tile([8 * 512, 256], u8)

            # ================= GNN phase =================
            with tc.tile_pool(name="gnn", bufs=4) as gpool, \
                 tc.tile_pool(name="gpsum", bufs=3, space="PSUM") as gpsum, \
                 tc.tile_pool(name="spsum", bufs=1, space="PSUM") as spsum:

                for (eflat, nodeTb_s, S_s, npos) in (
                        (ligedge_flat, nodeT_lig, S_lig, PLIG),
                        (recedge_flat, nodeT_rec, S_rec, PREC)):
                    nch = npos // KNB
                    starts = list(range(0, nch - nch % CH, CH))
                    sizes = [CH] * len(starts)
                    if nch % CH:
                        starts.append(nch - nch % CH)
                        sizes.append(nch % CH)
                    for c0, csz in zip(starts, sizes):
                        w = csz * KNB
                        et = gpool.tile([128, CH * KNB], f8, tag="edge")
                        nc.gpsimd.dma_start(
                            out=et[0:64, 0:w],
                            in_=eflat[:, c0 * KNB:c0 * KNB + w])
                        hz = gpsum.tile([128, CH * KNB], f32, tag="hz")
                        for q0 in range(0, w, 512):
                            qw = min(512, w - q0)
                            nc.tensor.matmul(
                                hz[:, q0:q0 + qw],
                                WETb_s[0:64, :],
                                et[0:64, q0:q0 + qw],
                                start=True, stop=False)
                            rhs = nodeTb_s[:, c0 + q0 // KNB:c0 + (q0 + qw) // KNB]
                            rhs = rhs.unsqueeze(2).broadcast_to(
                                [128, qw // KNB, KNB])
                            nc.tensor.matmul(
                                hz[:, q0:q0 + qw],
                                WNT_s[:], rhs,
                                start=False, stop=True)
                        zt = gpool.tile([128, CH * KNB], bf16, tag="zt")
                        nc.scalar.activation(zt[:, 0:w], hz[:, 0:w], AF.Tanh,
                                             bias=gnnbias_s[:, 0:1])
                        ztr = zt[:, 0:w].rearrange("p (n k) -> p n k", k=KNB)
                        with nc.allow_low_precision(
                                reason="S in bf16 feeds a bf16 matmul; "
                                       "2e-2 output tolerance"):
                            nc.vector.reduce_sum(
                                S_s[:, c0:c0 + csz], ztr,
                                axis=mybir.AxisListType.X)

                # ---- HOPI ----
                pp = spsum.tile([128, 512], f32, tag="sp")
                nc.tensor.matmul(pp[0:32, 0:NLIG], WlT_s[0:128, :], nodeT_lig[:],
                                 start=True, stop=False)
                nc.tensor.matmul(pp[0:32, 0:NLIG], WlT16_s[0:128, :], S_lig[:],
                                 start=False, stop=True)
                nc.vector.tensor_mul(plzA[0:32, :], pp[0:32, 0:260],
                                     plmask_s[0:32, :])
                nc.sync.dma_start(out=plzA[32:33, :], in_=plmaskrow_s[0:1, 0:260])

                pp2 = spsum.tile([128, 512], f32, tag="sp")
                nc.tensor.matmul(pp2[0:32, 0:NREC], WrT_s[0:128, :], nodeT_rec[:],
                                 start=True, stop=False)
                nc.tensor.matmul(pp2[0:32, 0:NREC], WrT16_s[0:128, :], S_rec[:],
                                 start=False, stop=True)
                nc.vector.memset(przA[0:32, 0:1], 0.0)
                nc.vector.memset(przA[0:32, 513:514], 0.0)
                nc.scalar.activation(przA[0:32, 1:513], pp2[0:32, 0:NREC], AF.Copy)

                # ---- U ----
                up = spsum.tile([128, 512], f32, tag="sp")
                for dl in range(3):
                    nc.tensor.matmul(up[0:32, 0:258],
                                     UW_s[0:32, 32 * dl:32 * dl + 32],
                                     plzA[0:32, dl:dl + 258],
                                     start=(dl == 0), stop=(dl == 2))
                nc.scalar.activation(U_sb[0:32, 0:258], up[0:32, 0:258], AF.Copy)

                # ---- c0 / c511 rows ----
                nc.sync.dma_start(out=A0AUG[0:32, :], in_=osrc("A0W"))
                nc.sync.dma_start(out=A511AUG[0:32, :], in_=osrc("A511W"))
                for which, (W1c_s, cc_s, dst) in enumerate(
                        ((W1c0_s, c0c_s, A0AUG), (W1c511_s, c511c_s, A511AUG))):
                    cp = spsum.tile([128, 512], f32, tag="sp")
                    for dl in range(3):
                        for t in range(2):
                            col = (1 + t) if which == 0 else (511 + t)
                            nc.tensor.matmul(
                                cp[0:1, 32 * dl:32 * dl + 32],
                                przA[0:32, col:col + 1],
                                W1c_s[0:32, 32 * (2 * dl + t):32 * (2 * dl + t) + 32],
                                start=(t == 0), stop=False)
                        nc.tensor.matmul(
                            cp[0:1, 32 * dl:32 * dl + 32],
                            ONE1_s[0:1, 0:1],
                            cc_s[0:1, 32 * dl:32 * dl + 32],
                            start=False, stop=True)
                    nc.scalar.activation(dst[32:33, 0:96], cp[0:1, 0:96], AF.Copy)

                # ---- Ucol0 / Ucol511 ----
                for AUG, dstu in ((A0AUG, Uc0_sb), (A511AUG, Uc511_sb)):
                    ucp = spsum.tile([128, 512], f32, tag="sp")
                    for dl in range(3):
                        nc.tensor.matmul(ucp[0:32, 0:258],
                                         AUG[0:33, 32 * dl:32 * dl + 32],
                                         plzA[0:33, dl:dl + 258],
                                         start=(dl == 0), stop=(dl == 2))
                    nc.scalar.activation(dstu[0:32, 0:258], ucp[0:32, 0:258], AF.Copy)

                # ---- V variants ----
                for VWx, VCx, vt in ((VW_s, VC_s, V_rep), (VWf_s, VCf_s, V_first),
                                     (VWq_s, VCq_s, V_qlast)):
                    vp = spsum.tile([128, 512], f32, tag="sp")
                    for dr in range(3):
                        nc.tensor.matmul(vp[:, 0:512],
                                         VWx[0:32, 128 * dr:128 * dr + 128],
                                         przA[0:32, dr:dr + 512],
                                         start=(dr == 0), stop=False)
                    nc.tensor.matmul(vp[:, 0:512], VCx[0:1, 0:128], ONESR_s[0:1, 0:512],
                                     start=False, stop=True)
                    nc.scalar.activation(vt[:], vp[:, 0:512], AF.Copy)

                # ---- u relayouts (i = 4s+j for P, 4s+2+j for Q) ----
                for (src, dstP, dstQ) in ((U_sb, uP, uQ), (Uc0_sb, uc0P, uc0Q),
                                          (Uc511_sb, uc511P, uc511Q)):
                    srcr = src[0:32, 0:260].rearrange("c (s f) -> c s f", f=4)
                    for j in range(4):
                        nc.sync.dma_start(out=dstP[32 * j:32 * j + 32, 0:64],
                                          in_=srcr[:, 0:64, j])
                    for j in range(2):
                        nc.sync.dma_start(out=dstQ[32 * j:32 * j + 32, 0:64],
                                          in_=srcr[:, 0:64, 2 + j])
                    for j in range(2, 4):
                        nc.sync.dma_start(out=dstQ[32 * j:32 * j + 32, 0:64],
                                          in_=srcr[:, 1:65, j - 2])
                for (t, col, rm) in ((uP, 0, rmP0_s), (uc0P, 0, rmP0_s),
                                     (uc511P, 0, rmP0_s), (uQ, 63, rmQ63_s),
                                     (uc0Q, 63, rmQ63_s), (uc511Q, 63, rmQ63_s)):
                    nc.vector.tensor_mul(t[:, col:col + 1], t[:, col:col + 1], rm[:])

            # ================= conv pipeline =================
            with tc.tile_pool(name="x1", bufs=3) as x1pool, \
                 tc.tile_pool(name="x2", bufs=3) as x2pool, \
                 tc.tile_pool(name="osb", bufs=2) as opool, \
                 tc.tile_pool(name="cpsum", bufs=4, space="PSUM") as cpsum, \
                 tc.tile_pool(name="c3ps", bufs=2, space="PSUM") as c3psum:

                for k in range(NSTRIP):
                    x1P = x1pool.tile([128, 8 * 514], bf16, tag="x1P")
                    x1Q = x1pool.tile([128, 8 * 514], bf16, tag="x1Q")
                    for s in range(8):
                        sg = 8 * k + s
                        for (tile_, uu, Vgen, is_edge, rm) in (
                                (x1P, uP, V_first if sg == 0 else V_rep, sg == 0, rmP0_s),
                                (x1Q, uQ, V_qlast if sg == 63 else V_rep, sg == 63, rmQ63_s)):
                            dst = tile_[:, s * 514 + 1:s * 514 + 513]
                            bias_ap = uu[:, sg:sg + 1]
                            if is_edge:
                                nc.scalar.activation(dst, Vgen[:], AF.Relu,
                                                     bias=bias_ap, scale=rm[:])
                            elif s % 3 == 0:
                                nc.scalar.activation(dst, Vgen[:], AF.Relu, bias=bias_ap)
                            else:
                                nc.vector.tensor_scalar(dst, Vgen[:], bias_ap, 0.0,
                                                        ALU.add, ALU.max)
                    for tile_, ucol0, ucol511 in ((x1P, uc0P, uc511P), (x1Q, uc0Q, uc511Q)):
                        tr = tile_[:].rearrange("p (s c) -> p s c", c=514)
                        nc.vector.memset(tr[:, :, 0], 0.0)
                        nc.vector.memset(tr[:, :, 513], 0.0)
                        nc.vector.tensor_scalar(tr[:, :, 1], ucol0[:, 8 * k:8 * k + 8],
                                                0.0, None, ALU.max)
                        nc.vector.tensor_scalar(tr[:, :, 512], ucol511[:, 8 * k:8 * k + 8],
                                                0.0, None, ALU.max)

                    x2 = x2pool.tile([128, 8 * 512], bf16, tag="x2")
                    for s in range(8):
                        c2 = cpsum.tile([128, 512], f32, tag="c2")
                        for dr in range(3):
                            wp0 = W2P0_s[:, 32 * dr:32 * dr + 32]
                            wp1 = W2P1_s[:, 32 * dr:32 * dr + 32]
                            rhsP = x1P[:, s * 514 + dr:s * 514 + dr + 512]
                            rhsQ = x1Q[:, s * 514 + dr:s * 514 + dr + 512]
                            st, sp_ = (dr == 0), (dr == 2)
                            nc.tensor.matmul(c2[0:32, :], wp0, rhsP, start=st, stop=sp_,
                                             tile_position=(0, 0), skip_group_check=True)
                            nc.tensor.matmul(c2[32:64, :], wp1, rhsP, start=st, stop=sp_,
                                             tile_position=(0, 32), skip_group_check=True)
                            nc.tensor.matmul(c2[64:96, :], wp0, rhsQ, start=st, stop=sp_,
                                             tile_position=(0, 64), skip_group_check=True)
                            nc.tensor.matmul(c2[96:128, :], wp1, rhsQ, start=st, stop=sp_,
                                             tile_position=(0, 96), skip_group_check=True)
                        dst2 = x2[:, s * 512:(s + 1) * 512]
                        if s % 3 != 2:
                            nc.scalar.activation(dst2, c2[:], AF.Relu, bias=bc2rep_s[:, 0:1])
                        else:
                            nc.vector.tensor_scalar(dst2, c2[:], bc2rep_s[:, 0:1], 0.0,
                                                    ALU.add, ALU.max)

                    # conv3: logits transposed onto 128 partitions (r-slab on
                    # partitions, strip-row on free); undone host-side.
                    c3p = c3psum.tile([128, 128], f32, tag="c3")
                    for s in range(8):
                        xc = x2[:, s * 512:(s + 1) * 512]
                        for u in range(4):
                            nc.tensor.matmul(
                                c3p[:, 32 * u + 4 * s:32 * u + 4 * s + 4],
                                xc[:, 128 * u:128 * u + 128],
                                W3sel_s[:, 0:4], start=True, stop=True)
                    osg = opool.tile([128, 128], f32, tag="osg")
                    nc.scalar.activation(osg[:], c3p[:], AF.Sigmoid,
                                         bias=b3vec_s[:, 0:1])
                    osb = opool.tile([128, 128], u8, tag="osb")
                    with nc.allow_low_precision(
                            reason="sigmoid output quantized to 1/255 steps; "
                                   "2e-2 output tolerance"):
                        nc.vector.tensor_scalar(osb[:], osg[:], 255.0, None,
                                                ALU.mult)
                    # osb[p, 32u+4s+m] = sigmoid(logit[row=4s+m, r=128u+p])
                    osr = osb[:].rearrange("p (u c) -> p u c", c=32)
                    outr = stage[:].rearrange("(u p) g -> p u g", p=128)
                    nc.sync.dma_start(out=outr[:, :, 32 * k:32 * k + 32],
                                      in_=osr)

            nc.gpsimd.collective_compute(
                "AllGather", mybir.AluOpType.bypass,
                replica_groups=[[0, 1, 2, 3, 4, 5, 6, 7]],
                ins=[stage.opt()], outs=[gout.opt()])
            nc.gpsimd.dma_start(out=out[:], in_=gout[:])

    nc.compile()
    return nc


def _get_runner():
    """Build (once) the cached jitted SPMD callable + on-device zeros maker."""
    if "runner" in _CACHE:
        return _CACHE["runner"]

    import jax
    import jax.numpy as jnp
    import concourse.mybir as mybir
    from concourse.bass2jax import (install_neuronx_cc_hook, _bass_exec_p,
                                    partition_id_tensor)
    from jax.sharding import Mesh, PartitionSpec, NamedSharding
    from jax.experimental.shard_map import shard_map

    nc = _build_program()
    install_neuronx_cc_hook()

    partition_name = nc.partition_id_tensor.name if nc.partition_id_tensor else None
    in_names, out_names, out_avals = [], [], []
    for alloc in nc.m.functions[0].allocations:
        if not isinstance(alloc, mybir.MemoryLocationSet):
            continue
        name = alloc.memorylocations[0].name
        if alloc.kind == "ExternalInput":
            if name != partition_name:
                in_names.append(name)
        elif alloc.kind == "ExternalOutput":
            out_names.append(name)
            out_avals.append(jax.core.ShapedArray(tuple(alloc.tensor_shape),
                                                  mybir.dt.np(alloc.dtype)))
    n_params, n_outs = len(in_names), len(out_avals)
    assert in_names == ["master"] and out_names == ["out"], (in_names, out_names)
    in_names_full = list(in_names) + list(out_names)
    if partition_name is not None:
        in_names_full.append(partition_name)

    def _body(*args):
        operands = list(args)
        if partition_name is not None:
            operands.append(partition_id_tensor())
        return tuple(_bass_exec_p.bind(
            *operands, out_avals=tuple(out_avals), in_names=tuple(in_names_full),
            out_names=tuple(out_names), lowering_input_output_aliases=(),
            sim_require_finite=True, sim_require_nnan=True, nc=nc))

    mesh = Mesh(np.asarray(jax.devices()[:8]), ("core",))
    sharded = jax.jit(
        shard_map(_body, mesh=mesh,
                  in_specs=(PartitionSpec("core"),) * (n_params + n_outs),
                  out_specs=(PartitionSpec("core"),) * n_outs, check_rep=False),
        donate_argnums=tuple(range(n_params, n_params + n_outs)),
        keep_unused=True)

    oshape = tuple(out_avals[0].shape)
    odtype = out_avals[0].dtype
    zmaker = jax.jit(lambda: jnp.zeros((8 * oshape[0],) + oshape[1:], odtype),
                     out_shardings=NamedSharding(mesh, PartitionSpec("core")))

    def run(master):
        """-> gathered output (4096, 256) uint8 from core 0 (one round trip)."""
        z = _CACHE.pop("znext", None)
        if z is None:
            z = zmaker()
        out_arrs = sharded(master, z)
        g = out_arrs[0]
        shard0 = min(g.addressable_shards, key=lambda s: s.index[0].start or 0)
        res = np.asarray(shard0.data)
        _CACHE["znext"] = zmaker()   # ready before the next call
        return res

    _CACHE["runner"] = run
    return run


def kernel(**inputs):
    run = _get_runner()
    packs = _host_prep(inputs)
    gathered = run(*packs)
    full = np.empty((B, L, R), np.float32)
    for core in range(8):
        b, h = core // 2, core % 2
        full[b, 256 * h:256 * h + 256, :] = \
            gathered[512 * core:512 * core + 512].astype(np.float32).T
    full *= (1.0 / 255.0)
    return full


# revision 16
# speedup vs baseline: 1.0203x; 1.0203x over previous
"""ProteinInterfacePrediction fused Bass kernel for 8 TRN2 NeuronCores.

Sharding: core c = (batch b = c//2, L-half h = c%2); each core computes its
(256, 512) output tile.

Decomposition (validated bit-level in numpy vs the jax reference):
  - GNN residual folded into HOPI: pl = Wl@nodeT + (Wl/16)@S, S = sum_k tanh(hn+he)
  - conv1 is rank-separable before relu: conv1(P) = U[co,l] + V[co,r] (+consts),
    boundary columns via mask-augmented 1-D convs, boundary rows via per-core
    flag-baked V-weight variants.
  - conv2 on TensorE: 4-input-row blocks on 128 partitions (K = 4rows x 32ci),
    stride-2 (P/Q dual layouts), 3 dr-taps, 4-way 32-column array tiling.
  - conv3 (1x1) + bias + sigmoid fused at the tail.

Dispatch layer (dominates wall-clock over the axon tunnel):
  - pair-shared data (weights + receptor features, identical for the two
    cores of a batch) uploaded once per pair as half-blobs and AllGather'd
    on-device over NeuronLink
  - remaining per-core tensors packed into ONE bf16 + ONE small f32 array
  - donated output buffers created on-device (no zero upload)
  - output written bf16 and AllGather'd on-device, so the host fetches a
    single 2 MB shard in one round trip
  - the jitted SPMD callable is built once and cached (no per-call
    recompile)
"""

import numpy as np
import ml_dtypes

B, L, R, KNB = 4, 512, 512, 16
DN, DE = 128, 64
NLIG = 260           # 256 + 2-row halo each side
NREC = 512
PLIG = NLIG * KNB
PREC = NREC * KNB
CH = 64              # gnn nodes per chunk
NSTRIP = 8

BF16 = ml_dtypes.bfloat16

# (name, rows, cols) — single source of truth for host + device.
PAIR_LAYOUT = [      # shared between the two cores of a batch
    ("WNT", 128, 128), ("WETb", 64, 128),
    ("WlT", 128, 32), ("WlT16", 128, 32), ("WrT", 128, 32), ("WrT16", 128, 32),
    ("UW", 32, 96), ("A0W", 32, 96), ("A511W", 32, 96),
    ("W1c0", 32, 192), ("W1c511", 32, 192),
    ("c0const", 1, 96), ("c511const", 1, 96),
    ("VW", 32, 384), ("VC", 1, 128),
    ("W2P0", 128, 96), ("W2P1", 128, 96), ("W3selb", 128, 4),
    ("ONE1", 1, 1), ("ONESR", 1, 512),
    ("recnodeT", 128, NREC),
]
CORE_LAYOUT = [      # genuinely per-core (h-dependent or ligand slice)
    ("VWfirst", 32, 384), ("VWqlast", 32, 384),
    ("VCfirst", 1, 128), ("VCqlast", 1, 128),
    ("plmask", 32, 260), ("plmaskrow", 1, 260),
    ("lignodeT", 128, NLIG),
]
PAIRF8_LAYOUT = [      # fp8 pair-shared (receptor edges)
    ("recedgeT", 64, PREC),
]
COREF8_LAYOUT = [      # fp8 per-core (ligand edge slice)
    ("ligedgeT", 64, PLIG),
]
F32_LAYOUT = [
    ("gnnbias", 128, 1), ("bc2rep", 128, 1), ("b3vec", 128, 1),
    ("rmP0", 128, 1), ("rmQ63", 128, 1),
]


def _offsets(layout):
    offs, off = {}, 0
    for name, r, c in layout:
        offs[name] = (off, r, c)
        off += r * c
    return offs, off


PAIR_OFFS, _PT = _offsets(PAIR_LAYOUT)          # bf16 elements
CORE_OFFS, CORE_TOTAL = _offsets(CORE_LAYOUT)   # bf16 elements
PAIRF8_OFFS, PAIRF8_TOTAL = _offsets(PAIRF8_LAYOUT)   # fp8 elements
COREF8_OFFS, COREF8_TOTAL = _offsets(COREF8_LAYOUT)   # fp8 elements
F32_OFFS, F32_TOTAL = _offsets(F32_LAYOUT)      # f32 elements
FP8 = ml_dtypes.float8_e4m3

def _pad4(n):
    return (n + 3) & ~3

# byte layout of the pair blob: [bf16 region][fp8 region], halved for upload
PAIR_BF_BYTES = _pad4(2 * _PT)
PAIR_BYTES = PAIR_BF_BYTES + _pad4(PAIRF8_TOTAL)
PAIR_BYTES += (-PAIR_BYTES) % 8192   # halves stay 4096-aligned for 2-D DMA
PH_BYTES = PAIR_BYTES // 2
# byte layout of the master wire pack: [pair half][core bf16][core fp8][f32]
CORE_BF_OFF = PH_BYTES
CORE_F8_OFF = CORE_BF_OFF + _pad4(2 * CORE_TOTAL)
F32_OFF = CORE_F8_OFF + _pad4(COREF8_TOTAL)
NBYTES = F32_OFF + 4 * F32_TOTAL

_CACHE = {}


def _host_prep(inputs):
    """Pack inputs into (8, PAIR_HALF) + (8, CORE_TOTAL) bf16 + (8, F32_TOTAL) f32."""
    f32 = np.float32
    W1 = np.asarray(inputs['Wc1'], f32)
    W2 = np.asarray(inputs['Wc2'], f32)
    W3 = np.asarray(inputs['Wc3'], f32)[0, :, 0, 0]
    b1 = np.asarray(inputs['bc1'], f32)
    b2 = np.asarray(inputs['bc2'], f32)
    b3 = float(np.asarray(inputs['bc3'], f32)[0])
    Wp = np.asarray(inputs['Wp'], f32)
    bp = np.asarray(inputs['bp'], f32)
    Wl, Wr = Wp[:, :DN], Wp[:, DN:]
    WN = np.asarray(inputs['WN'], f32)
    bN = np.asarray(inputs['bN'], f32)
    WE = np.asarray(inputs['WE'], f32)
    bE = np.asarray(inputs['bE'], f32)

    A = W1.sum(axis=3)
    Wv = W1.sum(axis=2)
    cU = np.einsum('oidr,i->od', W1, bp)

    sh = {}
    sh['WNT'] = WN.T
    sh['WETb'] = WE.T
    sh['WlT'] = Wl.T
    sh['WlT16'] = (Wl / 16.0).T
    sh['WrT'] = Wr.T
    sh['WrT16'] = (Wr / 16.0).T

    def pack3(M):  # (co, ci, dl) -> [32, 96] of [ci, co] blocks
        out = np.zeros((32, 96), f32)
        for dl in range(3):
            out[:, 32 * dl:32 * dl + 32] = M[:, :, dl].T
        return out

    sh['UW'] = pack3(A)
    sh['A0W'] = pack3(W1[:, :, :, 1:].sum(axis=3))
    sh['A511W'] = pack3(W1[:, :, :, :2].sum(axis=3))

    W1c0 = np.zeros((32, 192), f32)
    W1c511 = np.zeros((32, 192), f32)
    for dl in range(3):
        for t, dr in enumerate((1, 2)):
            W1c0[:, 32 * (2 * dl + t):32 * (2 * dl + t) + 32] = W1[:, :, dl, dr].T
        for t, dr in enumerate((0, 1)):
            W1c511[:, 32 * (2 * dl + t):32 * (2 * dl + t) + 32] = W1[:, :, dl, dr].T
    sh['W1c0'], sh['W1c511'] = W1c0, W1c511

    c0c = np.zeros((1, 96), f32)
    c511c = np.zeros((1, 96), f32)
    for dl in range(3):
        c0c[0, 32 * dl:32 * dl + 32] = np.einsum('oid,i->o', W1[:, :, dl, 1:], bp)
        c511c[0, 32 * dl:32 * dl + 32] = np.einsum('oid,i->o', W1[:, :, dl, :2], bp)
    c0c[0, 32:64] += b1
    c511c[0, 32:64] += b1
    sh['c0const'], sh['c511const'] = c0c, c511c

    VW = np.zeros((32, 384), f32)
    for dr in range(3):
        blk = Wv[:, :, dr].T
        for j in range(4):
            VW[:, 128 * dr + 32 * j:128 * dr + 32 * j + 32] = blk
    sh['VW'] = VW
    vc = cU.sum(axis=1) + b1
    VC = np.tile(vc, 4).reshape(1, 128)
    sh['VC'] = VC

    W2P0 = np.zeros((128, 96), f32)
    W2P1 = np.zeros((128, 96), f32)
    for dr in range(3):
        for j in range(3):
            W2P0[32 * j:32 * j + 32, 32 * dr:32 * dr + 32] = W2[:, :, j, dr].T
        for j in range(1, 4):
            W2P1[32 * j:32 * j + 32, 32 * dr:32 * dr + 32] = W2[:, :, j - 1, dr].T
    sh['W2P0'], sh['W2P1'] = W2P0, W2P1

    W3sel = np.zeros((128, 4), f32)
    for j in range(4):
        W3sel[32 * j:32 * j + 32, j] = W3
    sh['W3selb'] = W3sel
    sh['ONE1'] = np.ones((1, 1), f32)
    sh['ONESR'] = np.ones((1, 512), f32)

    shf = {}
    shf['gnnbias'] = (bN + bE).reshape(DN, 1)
    shf['bc2rep'] = np.tile(b2, 4).reshape(128, 1)
    shf['b3vec'] = np.full((128, 1), b3, f32)

    # big features: convert to bf16 once, per batch, then slice per core
    lig_nf = np.asarray(inputs['ligand_node_features'], f32).astype(BF16)
    lig_ef = np.asarray(inputs['ligand_edge_features'], f32).astype(FP8)
    rec_nf = np.asarray(inputs['receptor_node_features'], f32).astype(BF16)
    rec_ef = np.asarray(inputs['receptor_edge_features'], f32).astype(FP8)
    lig_nfT = [np.ascontiguousarray(lig_nf[b].T) for b in range(B)]      # (128, L)
    lig_efT = [np.ascontiguousarray(lig_ef[b].reshape(L * KNB, DE).T)
               for b in range(B)]                                        # (64, L*K)
    rec_nfT = [np.ascontiguousarray(rec_nf[b].T) for b in range(B)]
    rec_efT = [np.ascontiguousarray(rec_ef[b].reshape(R * KNB, DE).T)
               for b in range(B)]

    master = np.zeros((8, NBYTES), np.uint8)
    packb = np.zeros((8, CORE_TOTAL), BF16)
    packb8 = np.zeros((8, COREF8_TOTAL), FP8)
    packf = np.zeros((8, F32_TOTAL), np.float32)

    for b in range(B):
        blobby = np.zeros(PAIR_BYTES, np.uint8)
        blob = blobby[:2 * _PT].view(BF16)

        def putp(name, arr):
            off, r, c = PAIR_OFFS[name]
            a = np.asarray(arr)
            assert a.shape == (r, c), (name, a.shape, (r, c))
            blob[off:off + r * c] = a.astype(BF16, copy=False).ravel()

        for name in ('WNT', 'WETb', 'WlT', 'WlT16', 'WrT', 'WrT16', 'UW',
                     'A0W', 'A511W', 'W1c0', 'W1c511', 'c0const', 'c511const',
                     'VW', 'VC', 'W2P0', 'W2P1', 'W3selb', 'ONE1', 'ONESR'):
            putp(name, sh[name])
        putp('recnodeT', rec_nfT[b])
        o8, r8, c8 = PAIRF8_OFFS['recedgeT']
        blobby[PAIR_BF_BYTES + o8:PAIR_BF_BYTES + o8 + r8 * c8] = \
            rec_efT[b].reshape(-1).view(np.uint8)
        master[2 * b, :PH_BYTES] = blobby[:PH_BYTES]
        master[2 * b + 1, :PH_BYTES] = blobby[PH_BYTES:]

    for core in range(8):
        b, h = core // 2, core % 2
        lo = 256 * h - 2

        def putb(name, arr):
            off, r, c = CORE_OFFS[name]
            a = np.asarray(arr)
            assert a.shape == (r, c), (name, a.shape, (r, c))
            packb[core, off:off + r * c] = a.astype(BF16, copy=False).ravel()

        def putf(name, arr):
            off, r, c = F32_OFFS[name]
            packf[core, off:off + r * c] = np.asarray(arr, np.float32).ravel()

        for name in ('gnnbias', 'bc2rep', 'b3vec'):
            putf(name, shf[name])

        lig_node = np.zeros((128, NLIG), BF16)
        lig_edge = np.zeros((64, PLIG), FP8)
        g0, g1 = max(lo, 0), min(lo + 260, L)
        lig_node[:, g0 - lo:g1 - lo] = lig_nfT[b][:, g0:g1]
        lig_edge[:, (g0 - lo) * KNB:(g1 - lo) * KNB] = \
            lig_efT[b][:, g0 * KNB:g1 * KNB]
        putb('lignodeT', lig_node)
        o8, r8, c8 = COREF8_OFFS['ligedgeT']
        packb8[core, o8:o8 + r8 * c8] = lig_edge.ravel()

        plmask = np.array([1.0 if 0 <= lo + i < L else 0.0 for i in range(260)],
                          np.float32)
        putb('plmask', np.tile(plmask.reshape(1, 260), (32, 1)))
        putb('plmaskrow', plmask.reshape(1, 260))

        flag0 = 1.0 if h == 0 else 0.0
        flag1 = 1.0 if h == 1 else 0.0
        VWfirst = sh['VW'].copy()
        VWqlast = sh['VW'].copy()
        for dr in range(3):
            VWfirst[:, 128 * dr + 32:128 * dr + 64] -= flag0 * W1[:, :, 0, dr].T
            VWqlast[:, 128 * dr + 64:128 * dr + 96] -= flag1 * W1[:, :, 2, dr].T
        putb('VWfirst', VWfirst)
        putb('VWqlast', VWqlast)

        VCfirst, VCqlast = VC.copy(), VC.copy()
        VCfirst[0, 32:64] -= flag0 * cU[:, 0]
        VCqlast[0, 64:96] -= flag1 * cU[:, 2]
        putb('VCfirst', VCfirst)
        putb('VCqlast', VCqlast)

        rmP0 = np.ones((128, 1), f32)
        rmQ63 = np.ones((128, 1), f32)
        for j in range(4):
            if not (0 <= 256 * h + (j - 1) < L):
                rmP0[32 * j:32 * j + 32] = 0.0
            if not (0 <= 256 * h + (253 + j) < L):
                rmQ63[32 * j:32 * j + 32] = 0.0
        putf('rmP0', rmP0)
        putf('rmQ63', rmQ63)
        master[core, CORE_BF_OFF:CORE_BF_OFF + 2 * CORE_TOTAL] = \
            packb[core].view(np.uint8)
        master[core, CORE_F8_OFF:CORE_F8_OFF + COREF8_TOTAL] = \
            packb8[core].view(np.uint8)
        master[core, F32_OFF:F32_OFF + 4 * F32_TOTAL] = \
            packf[core].view(np.uint8)
    return (master,)


def _build_program():
    import concourse.bacc as bacc
    import concourse.mybir as mybir
    from concourse.tile import TileContext

    dt = mybir.dt
    f32, bf16 = dt.float32, dt.bfloat16
    AF = mybir.ActivationFunctionType
    ALU = mybir.AluOpType

    nc = bacc.Bacc("TRN2", target_bir_lowering=False, debug=False, num_devices=8)

    f8 = dt.float8e4
    u8 = dt.uint8
    masterd = nc.dram_tensor("master", [1, NBYTES], u8, kind="ExternalInput")
    out = nc.dram_tensor("out", [8 * 512, 256], u8, kind="ExternalOutput")

    with TileContext(nc) as tc:
        with tc.tile_pool(name="const", bufs=1) as cpool, \
             tc.tile_pool(name="dstage", bufs=1, space="DRAM") as dpool:

            # ---- pair AllGather: half-blob -> full shared blob (bytes) ----
            pbounce = dpool.tile([1, PH_BYTES], u8)
            gpair = dpool.tile([1, PAIR_BYTES], u8)
            nc.gpsimd.dma_start(
                out=pbounce[:].rearrange("o (r c) -> (o r) c", c=4096),
                in_=masterd[0:1, 0:PH_BYTES].rearrange(
                    "o (r c) -> (o r) c", c=4096))
            nc.gpsimd.collective_compute(
                "AllGather", mybir.AluOpType.bypass,
                replica_groups=[[0, 1], [2, 3], [4, 5], [6, 7]],
                ins=[pbounce[:]],
                outs=[gpair[:].rearrange("o (a b) -> (o a) b", a=2)])

            def gsrc(name):
                off, r, c = PAIR_OFFS[name]
                return gpair[0:1, 2 * off:2 * (off + r * c)].bitcast(
                    bf16).rearrange("o (r c) -> (o r) c", c=c)

            def bsrc(name):
                off, r, c = CORE_OFFS[name]
                bo = CORE_BF_OFF + 2 * off
                return masterd[0:1, bo:bo + 2 * r * c].bitcast(
                    bf16).rearrange("o (r c) -> (o r) c", c=c)

            def fsrc(name):
                off, r, c = F32_OFFS[name]
                bo = F32_OFF + 4 * off
                return masterd[0:1, bo:bo + 4 * r * c].bitcast(
                    f32).rearrange("o (r c) -> (o r) c", c=c)

            def ctile(name, dtype=bf16, src=None, offs=None):
                off, r, c = offs[name]
                t = cpool.tile([128, c], dtype, tag=f"c_{name}")
                nc.sync.dma_start(out=t[0:r, 0:c], in_=src(name))
                return t

            def ptile(name):
                return ctile(name, src=gsrc, offs=PAIR_OFFS)

            def btile(name):
                return ctile(name, src=bsrc, offs=CORE_OFFS)

            def ftile(name):
                return ctile(name, dtype=f32, src=fsrc, offs=F32_OFFS)

            WNT_s = ptile("WNT")
            WETb_s = cpool.tile([128, 128], f8, tag="c_WETb")
            nc.gpsimd.dma_start(out=WETb_s[0:64, 0:128], in_=gsrc("WETb"))
            gnnbias_s = ftile("gnnbias")
            WlT_s, WlT16_s = ptile("WlT"), ptile("WlT16")
            WrT_s, WrT16_s = ptile("WrT"), ptile("WrT16")
            UW_s = ptile("UW")
            W1c0_s, W1c511_s = ptile("W1c0"), ptile("W1c511")
            c0c_s, c511c_s = ptile("c0const"), ptile("c511const")
            VW_s, VWf_s, VWq_s = ptile("VW"), btile("VWfirst"), btile("VWqlast")
            VC_s, VCf_s, VCq_s = ptile("VC"), btile("VCfirst"), btile("VCqlast")
            W2P0_s, W2P1_s = ptile("W2P0"), ptile("W2P1")
            W3sel_s = ptile("W3selb")
            bc2rep_s, b3vec_s = ftile("bc2rep"), ftile("b3vec")
            ONE1_s, ONESR_s = ptile("ONE1"), ptile("ONESR")
            plmask_s = btile("plmask")
            plmaskrow_s = btile("plmaskrow")
            rmP0_s, rmQ63_s = ftile("rmP0"), ftile("rmQ63")
            nodeT_lig = btile("lignodeT")
            nodeT_rec = ptile("recnodeT")

            lo8 = CORE_F8_OFF + COREF8_OFFS["ligedgeT"][0]
            ligedge_flat = masterd[0:1, lo8:lo8 + 64 * PLIG].bitcast(
                f8).rearrange("o (r c) -> (o r) c", c=PLIG)
            ro8 = PAIR_BF_BYTES + PAIRF8_OFFS["recedgeT"][0]
            recedge_flat = gpair[0:1, ro8:ro8 + 64 * PREC].bitcast(
                f8).rearrange("o (r c) -> (o r) c", c=PREC)

            S_lig = cpool.tile([128, NLIG], bf16)
            S_rec = cpool.tile([128, NREC], bf16)
            plzA = cpool.tile([128, 260], bf16)     # rows 0-31 plz, row 32 mask
            przA = cpool.tile([128, 514], bf16)
            U_sb = cpool.tile([128, 260], f32)
            Uc0_sb = cpool.tile([128, 260], f32)
            Uc511_sb = cpool.tile([128, 260], f32)
            A0AUG = cpool.tile([128, 96], bf16)
            A511AUG = cpool.tile([128, 96], bf16)
            V_rep = cpool.tile([128, 512], f32)
            V_first = cpool.tile([128, 512], f32)
            V_qlast = cpool.tile([128, 512], f32)
            uP = cpool.tile([128, 64], f32, tag="uP")
            uQ = cpool.tile([128, 64], f32, tag="uQ")
            uc0P = cpool.tile([128, 64], f32, tag="uc0P")
            uc0Q = cpool.tile([128, 64], f32, tag="uc0Q")
            uc511P = cpool.tile([128, 64], f32, tag="uc511P")
            uc511Q = cpool.tile([128, 64], f32, tag="uc511Q")

            stage = dpool.tile([512, 256], u8)
            gout = dpool.tile([8 * 512, 256], u8)

            # ================= GNN phase =================
            with tc.tile_pool(name="gnn", bufs=4) as gpool, \
                 tc.tile_pool(name="gpsum", bufs=3, space="PSUM") as gpsum, \
                 tc.tile_pool(name="spsum", bufs=1, space="PSUM") as spsum:

                for (eflat, nodeTb_s, S_s, npos) in (
                        (ligedge_flat, nodeT_lig, S_lig, PLIG),
                        (recedge_flat, nodeT_rec, S_rec, PREC)):
                    nch = npos // KNB
                    starts = list(range(0, nch - nch % CH, CH))
                    sizes = [CH] * len(starts)
                    if nch % CH:
                        starts.append(nch - nch % CH)
                        sizes.append(nch % CH)
                    for c0, csz in zip(starts, sizes):
                        w = csz * KNB
                        et = gpool.tile([128, CH * KNB], f8, tag="edge")
                        nc.gpsimd.dma_start(
                            out=et[0:64, 0:w],
                            in_=eflat[:, c0 * KNB:c0 * KNB + w])
                        hz = gpsum.tile([128, CH * KNB], f32, tag="hz")
                        for q0 in range(0, w, 512):
                            qw = min(512, w - q0)
                            nc.tensor.matmul(
                                hz[:, q0:q0 + qw],
                                WETb_s[0:64, :],
                                et[0:64, q0:q0 + qw],
                                start=True, stop=False)
                            rhs = nodeTb_s[:, c0 + q0 // KNB:c0 + (q0 + qw) // KNB]
                            rhs = rhs.unsqueeze(2).broadcast_to(
                                [128, qw // KNB, KNB])
                            nc.tensor.matmul(
                                hz[:, q0:q0 + qw],
                                WNT_s[:], rhs,
                                start=False, stop=True)
                        zt = gpool.tile([128, CH * KNB], bf16, tag="zt")
                        nc.scalar.activation(zt[:, 0:w], hz[:, 0:w], AF.Tanh,
                                             bias=gnnbias_s[:, 0:1])
                        ztr = zt[:, 0:w].rearrange("p (n k) -> p n k", k=KNB)
                        with nc.allow_low_precision(
                                reason="S in bf16 feeds a bf16 matmul; "
                                       "2e-2 output tolerance"):
                            nc.vector.reduce_sum(
                                S_s[:, c0:c0 + csz], ztr,
                                axis=mybir.AxisListType.X)

                # ---- HOPI ----
                pp = spsum.tile([128, 512], f32, tag="sp")
                nc.tensor.matmul(pp[0:32, 0:NLIG], WlT_s[0:128, :], nodeT_lig[:],
                                 start=True, stop=False)
                nc.tensor.matmul(pp[0:32, 0:NLIG], WlT16_s[0:128, :], S_lig[:],
                                 start=False, stop=True)
                nc.vector.tensor_mul(plzA[0:32, :], pp[0:32, 0:260],
                                     plmask_s[0:32, :])
                nc.sync.dma_start(out=plzA[32:33, :], in_=plmaskrow_s[0:1, 0:260])

                pp2 = spsum.tile([128, 512], f32, tag="sp")
                nc.tensor.matmul(pp2[0:32, 0:NREC], WrT_s[0:128, :], nodeT_rec[:],
                                 start=True, stop=False)
                nc.tensor.matmul(pp2[0:32, 0:NREC], WrT16_s[0:128, :], S_rec[:],
                                 start=False, stop=True)
                nc.vector.memset(przA[0:32, 0:1], 0.0)
                nc.vector.memset(przA[0:32, 513:514], 0.0)
                nc.scalar.activation(przA[0:32, 1:513], pp2[0:32, 0:NREC], AF.Copy)

                # ---- U ----
                up = spsum.tile([128, 512], f32, tag="sp")
                for dl in range(3):
                    nc.tensor.matmul(up[0:32, 0:258],
                                     UW_s[0:32, 32 * dl:32 * dl + 32],
                                     plzA[0:32, dl:dl + 258],
                                     start=(dl == 0), stop=(dl == 2))
                nc.scalar.activation(U_sb[0:32, 0:258], up[0:32, 0:258], AF.Copy)

                # ---- c0 / c511 rows ----
                nc.sync.dma_start(out=A0AUG[0:32, :], in_=gsrc("A0W"))
                nc.sync.dma_start(out=A511AUG[0:32, :], in_=gsrc("A511W"))
                for which, (W1c_s, cc_s, dst) in enumerate(
                        ((W1c0_s, c0c_s, A0AUG), (W1c511_s, c511c_s, A511AUG))):
                    cp = spsum.tile([128, 512], f32, tag="sp")
                    for dl in range(3):
                        for t in range(2):
                            col = (1 + t) if which == 0 else (511 + t)
                            nc.tensor.matmul(
                                cp[0:1, 32 * dl:32 * dl + 32],
                                przA[0:32, col:col + 1],
                                W1c_s[0:32, 32 * (2 * dl + t):32 * (2 * dl + t) + 32],
                                start=(t == 0), stop=False)
                        nc.tensor.matmul(
                            cp[0:1, 32 * dl:32 * dl + 32],
                            ONE1_s[0:1, 0:1],
                            cc_s[0:1, 32 * dl:32 * dl + 32],
                            start=False, stop=True)
                    nc.scalar.activation(dst[32:33, 0:96], cp[0:1, 0:96], AF.Copy)

                # ---- Ucol0 / Ucol511 ----
                for AUG, dstu in ((A0AUG, Uc0_sb), (A511AUG, Uc511_sb)):
                    ucp = spsum.tile([128, 512], f32, tag="sp")
                    for dl in range(3):
                        nc.tensor.matmul(ucp[0:32, 0:258],
                                         AUG[0:33, 32 * dl:32 * dl + 32],
                                         plzA[0:33, dl:dl + 258],
                                         start=(dl == 0), stop=(dl == 2))
                    nc.scalar.activation(dstu[0:32, 0:258], ucp[0:32, 0:258], AF.Copy)

                # ---- V variants ----
                for VWx, VCx, vt in ((VW_s, VC_s, V_rep), (VWf_s, VCf_s, V_first),
                                     (VWq_s, VCq_s, V_qlast)):
                    vp = spsum.tile([128, 512], f32, tag="sp")
                    for dr in range(3):
                        nc.tensor.matmul(vp[:, 0:512],
                                         VWx[0:32, 128 * dr:128 * dr + 128],
                                         przA[0:32, dr:dr + 512],
                                         start=(dr == 0), stop=False)
                    nc.tensor.matmul(vp[:, 0:512], VCx[0:1, 0:128], ONESR_s[0:1, 0:512],
                                     start=False, stop=True)
                    nc.scalar.activation(vt[:], vp[:, 0:512], AF.Copy)

                # ---- u relayouts (i = 4s+j for P, 4s+2+j for Q) ----
                for (src, dstP, dstQ) in ((U_sb, uP, uQ), (Uc0_sb, uc0P, uc0Q),
                                          (Uc511_sb, uc511P, uc511Q)):
                    srcr = src[0:32, 0:260].rearrange("c (s f) -> c s f", f=4)
                    for j in range(4):
                        nc.sync.dma_start(out=dstP[32 * j:32 * j + 32, 0:64],
                                          in_=srcr[:, 0:64, j])
                    for j in range(2):
                        nc.sync.dma_start(out=dstQ[32 * j:32 * j + 32, 0:64],
                                          in_=srcr[:, 0:64, 2 + j])
                    for j in range(2, 4):
                        nc.sync.dma_start(out=dstQ[32 * j:32 * j + 32, 0:64],
                                          in_=srcr[:, 1:65, j - 2])
                for (t, col, rm) in ((uP, 0, rmP0_s), (uc0P, 0, rmP0_s),
                                     (uc511P, 0, rmP0_s), (uQ, 63, rmQ63_s),
                                     (uc0Q, 63, rmQ63_s), (uc511Q, 63, rmQ63_s)):
                    nc.vector.tensor_mul(t[:, col:col + 1], t[:, col:col + 1], rm[:])

            # ================= conv pipeline =================
            with tc.tile_pool(name="x1", bufs=3) as x1pool, \
                 tc.tile_pool(name="x2", bufs=3) as x2pool, \
                 tc.tile_pool(name="osb", bufs=2) as opool, \
                 tc.tile_pool(name="cpsum", bufs=4, space="PSUM") as cpsum, \
                 tc.tile_pool(name="c3ps", bufs=2, space="PSUM") as c3psum:

                for k in range(NSTRIP):
                    x1P = x1pool.tile([128, 8 * 514], bf16, tag="x1P")
                    x1Q = x1pool.tile([128, 8 * 514], bf16, tag="x1Q")
                    for s in range(8):
                        sg = 8 * k + s
                        for (tile_, uu, Vgen, is_edge, rm) in (
                                (x1P, uP, V_first if sg == 0 else V_rep, sg == 0, rmP0_s),
                                (x1Q, uQ, V_qlast if sg == 63 else V_rep, sg == 63, rmQ63_s)):
                            dst = tile_[:, s * 514 + 1:s * 514 + 513]
                            bias_ap = uu[:, sg:sg + 1]
                            if is_edge:
                                nc.scalar.activation(dst, Vgen[:], AF.Relu,
                                                     bias=bias_ap, scale=rm[:])
                            elif s % 3 == 0:
                                nc.scalar.activation(dst, Vgen[:], AF.Relu, bias=bias_ap)
                            else:
                                nc.vector.tensor_scalar(dst, Vgen[:], bias_ap, 0.0,
                                                        ALU.add, ALU.max)
                    for tile_, ucol0, ucol511 in ((x1P, uc0P, uc511P), (x1Q, uc0Q, uc511Q)):
                        tr = tile_[:].rearrange("p (s c) -> p s c", c=514)
                        nc.vector.memset(tr[:, :, 0], 0.0)
                        nc.vector.memset(tr[:, :, 513], 0.0)
                        nc.vector.tensor_scalar(tr[:, :, 1], ucol0[:, 8 * k:8 * k + 8],
                                                0.0, None, ALU.max)
                        nc.vector.tensor_scalar(tr[:, :, 512], ucol511[:, 8 * k:8 * k + 8],
                                                0.0, None, ALU.max)

                    x2 = x2pool.tile([128, 8 * 512], bf16, tag="x2")
                    for s in range(8):
                        c2 = cpsum.tile([128, 512], f32, tag="c2")
                        for dr in range(3):
                            wp0 = W2P0_s[:, 32 * dr:32 * dr + 32]
                            wp1 = W2P1_s[:, 32 * dr:32 * dr + 32]
                            rhsP = x1P[:, s * 514 + dr:s * 514 + dr + 512]
                            rhsQ = x1Q[:, s * 514 + dr:s * 514 + dr + 512]
                            st, sp_ = (dr == 0), (dr == 2)
                            nc.tensor.matmul(c2[0:32, :], wp0, rhsP, start=st, stop=sp_,
                                             tile_position=(0, 0), skip_group_check=True)
                            nc.tensor.matmul(c2[32:64, :], wp1, rhsP, start=st, stop=sp_,
                                             tile_position=(0, 32), skip_group_check=True)
                            nc.tensor.matmul(c2[64:96, :], wp0, rhsQ, start=st, stop=sp_,
                                             tile_position=(0, 64), skip_group_check=True)
                            nc.tensor.matmul(c2[96:128, :], wp1, rhsQ, start=st, stop=sp_,
                                             tile_position=(0, 96), skip_group_check=True)
                        dst2 = x2[:, s * 512:(s + 1) * 512]
                        if s % 3 != 2:
                            nc.scalar.activation(dst2, c2[:], AF.Relu, bias=bc2rep_s[:, 0:1])
                        else:
                            nc.vector.tensor_scalar(dst2, c2[:], bc2rep_s[:, 0:1], 0.0,
                                                    ALU.add, ALU.max)

                    # conv3: logits transposed onto 128 partitions (r-slab on
                    # partitions, strip-row on free); undone host-side.
                    c3p = c3psum.tile([128, 128], f32, tag="c3")
                    for s in range(8):
                        xc = x2[:, s * 512:(s + 1) * 512]
                        for u in range(4):
                            nc.tensor.matmul(
                                c3p[:, 32 * u + 4 * s:32 * u + 4 * s + 4],
                                xc[:, 128 * u:128 * u + 128],
                                W3sel_s[:, 0:4], start=True, stop=True)
                    osg = opool.tile([128, 128], f32, tag="osg")
                    nc.scalar.activation(osg[:], c3p[:], AF.Sigmoid,
                                         bias=b3vec_s[:, 0:1])
                    osb = opool.tile([128, 128], u8, tag="osb")
                    with nc.allow_low_precision(
                            reason="sigmoid output quantized to 1/255 steps; "
                                   "2e-2 output tolerance"):
                        nc.vector.tensor_scalar(osb[:], osg[:], 255.0, None,
                                                ALU.mult)
                    # osb[p, 32u+4s+m] = sigmoid(logit[row=4s+m, r=128u+p])
                    osr = osb[:].rearrange("p (u c) -> p u c", c=32)
                    outr = stage[:].rearrange("(u p) g -> p u g", p=128)
                    nc.sync.dma_start(out=outr[:, :, 32 * k:32 * k + 32],
                                      in_=osr)

            nc.gpsimd.collective_compute(
                "AllGather", mybir.AluOpType.bypass,
                replica_groups=[[0, 1, 2, 3, 4, 5, 6, 7]],
                ins=[stage.opt()], outs=[gout.opt()])
            nc.gpsimd.dma_start(out=out[:], in_=gout[:])

    nc.compile()
    return nc


def _get_runner():
    """Build (once) the cached jitted SPMD callable + on-device zeros maker."""
    if "runner" in _CACHE:
        return _CACHE["runner"]

    import jax
    import jax.numpy as jnp
    import concourse.mybir as mybir
    from concourse.bass2jax import (install_neuronx_cc_hook, _bass_exec_p,
                                    partition_id_tensor)
    from jax.sharding import Mesh, PartitionSpec, NamedSharding
    from jax.experimental.shard_map import shard_map

    nc = _build_program()
    install_neuronx_cc_hook()

    partition_name = nc.partition_id_tensor.name if nc.partition_id_tensor else None
    in_names, out_names, out_avals = [], [], []
    for alloc in nc.m.functions[0].allocations:
        if not isinstance(alloc, mybir.MemoryLocationSet):
            continue
        name = alloc.memorylocations[0].name
        if alloc.kind == "ExternalInput":
            if name != partition_name:
                in_names.append(name)
        elif alloc.kind == "ExternalOutput":
            out_names.append(name)
            out_avals.append(jax.core.ShapedArray(tuple(alloc.tensor_shape),
                                                  mybir.dt.np(alloc.dtype)))
    n_params, n_outs = len(in_names), len(out_avals)
    assert in_names == ["master"] and out_names == ["out"], (in_names, out_names)
    in_names_full = list(in_names) + list(out_names)
    if partition_name is not None:
        in_names_full.append(partition_name)

    def _body(*args):
        operands = list(args)
        if partition_name is not None:
            operands.append(partition_id_tensor())
        return tuple(_bass_exec_p.bind(
            *operands, out_avals=tuple(out_avals), in_names=tuple(in_names_full),
            out_names=tuple(out_names), lowering_input_output_aliases=(),
            sim_require_finite=True, sim_require_nnan=True, nc=nc))

    mesh = Mesh(np.asarray(jax.devices()[:8]), ("core",))
    sharded = jax.jit(
        shard_map(_body, mesh=mesh,
                  in_specs=(PartitionSpec("core"),) * (n_params + n_outs),
                  out_specs=(PartitionSpec("core"),) * n_outs, check_rep=False),
        donate_argnums=tuple(range(n_params, n_params + n_outs)),
        keep_unused=True)

    oshape = tuple(out_avals[0].shape)
    odtype = out_avals[0].dtype
    zmaker = jax.jit(lambda: jnp.zeros((8 * oshape[0],) + oshape[1:], odtype),
                     out_shardings=NamedSharding(mesh, PartitionSpec("core")))

    def run(master):
        """-> gathered output (4096, 256) uint8 from core 0 (one round trip)."""
        z = _CACHE.pop("znext", None)
        if z is None:
            z = zmaker()
        out_arrs = sharded(master, z)
        g = out_arrs[0]
        shard0 = min(g.addressable_shards, key=lambda s: s.index[0].start or 0)
        res = np.asarray(shard0.data)
        _CACHE["znext"] = zmaker()   # ready before the next call
        return res

    _CACHE["runner"] = run
    return run


def kernel(**inputs):
    run = _get_runner()
    packs = _host_prep(inputs)
    gathered = run(*packs)
    full = np.empty((B, L, R), np.float32)
    for core in range(8):
        b, h = core // 2, core % 2
        full[b, 256 * h:256 * h + 256, :] = \
            gathered[512 * core:512 * core + 512].astype(np.float32).T
    full *= (1.0 / 255.0)
    return full


# revision 17
# speedup vs baseline: 1.0734x; 1.0520x over previous
"""ProteinInterfacePrediction fused Bass kernel for 8 TRN2 NeuronCores.

Sharding: core c = (batch b = c//2, L-half h = c%2); each core computes its
(256, 512) output tile.

Decomposition (validated bit-level in numpy vs the jax reference):
  - GNN residual folded into HOPI: pl = Wl@nodeT + (Wl/16)@S, S = sum_k tanh(hn+he)
  - conv1 is rank-separable before relu: conv1(P) = U[co,l] + V[co,r] (+consts),
    boundary columns via mask-augmented 1-D convs, boundary rows via per-core
    flag-baked V-weight variants.
  - conv2 on TensorE: 4-input-row blocks on 128 partitions (K = 4rows x 32ci),
    stride-2 (P/Q dual layouts), 3 dr-taps, 4-way 32-column array tiling.
  - conv3 (1x1) + bias + sigmoid fused at the tail.

Dispatch layer (dominates wall-clock over the axon tunnel):
  - ALL wire data packed into ONE uint8 array per core (regions bitcast to
    bf16 / fp8e4m3 / f32 on device); edge features shipped fp8, nodes and
    weights bf16
  - replicated weights uploaded once as eighth-blobs (8-way AllGather on
    device); receptor features, identical for the two cores of a batch,
    uploaded once per pair as half-blobs (pair AllGather over NeuronLink)
  - donated output buffers created on-device (no zero upload), prefetched
    for the next call
  - output sigmoid quantized to uint8 (1/255 steps) and AllGather'd
    on-device, so the host fetches a single 1 MB shard in one round trip
  - the jitted SPMD callable is built once and cached (no per-call
    recompile)
"""

import numpy as np
import ml_dtypes

B, L, R, KNB = 4, 512, 512, 16
DN, DE = 128, 64
NLIG = 260           # 256 + 2-row halo each side
NREC = 512
PLIG = NLIG * KNB
PREC = NREC * KNB
CH = 64              # gnn nodes per chunk
NSTRIP = 8

BF16 = ml_dtypes.bfloat16

# (name, rows, cols) — single source of truth for host + device.
OCT_LAYOUT = [       # identical on all 8 cores (weights)
    ("WNT", 128, 128), ("WETb", 64, 128),
    ("WlT", 128, 32), ("WlT16", 128, 32), ("WrT", 128, 32), ("WrT16", 128, 32),
    ("UW", 32, 96), ("A0W", 32, 96), ("A511W", 32, 96),
    ("W1c0", 32, 192), ("W1c511", 32, 192),
    ("c0const", 1, 96), ("c511const", 1, 96),
    ("VW", 32, 384), ("VC", 1, 128),
    ("W2P0", 128, 96), ("W2P1", 128, 96), ("W3selb", 128, 4),
    ("ONE1", 1, 1), ("ONESR", 1, 512),
]
PAIR_LAYOUT = [      # shared between the two cores of a batch
    ("recnodeT", 128, NREC),
]
CORE_LAYOUT = [      # genuinely per-core (h-dependent or ligand slice)
    ("VWfirst", 32, 384), ("VWqlast", 32, 384),
    ("VCfirst", 1, 128), ("VCqlast", 1, 128),
    ("plmask", 32, 260), ("plmaskrow", 1, 260),
    ("lignodeT", 128, NLIG),
]
PAIRF8_LAYOUT = [      # fp8 pair-shared (receptor edges)
    ("recedgeT", 64, PREC),
]
COREF8_LAYOUT = [      # fp8 per-core (ligand edge slice)
    ("ligedgeT", 64, PLIG),
]
F32_LAYOUT = [
    ("gnnbias", 128, 1), ("bc2rep", 128, 1), ("b3vec", 128, 1),
    ("rmP0", 128, 1), ("rmQ63", 128, 1),
]


def _offsets(layout):
    offs, off = {}, 0
    for name, r, c in layout:
        offs[name] = (off, r, c)
        off += r * c
    return offs, off


OCT_OFFS, _OT = _offsets(OCT_LAYOUT)            # bf16 elements
PAIR_OFFS, _PT = _offsets(PAIR_LAYOUT)          # bf16 elements
CORE_OFFS, CORE_TOTAL = _offsets(CORE_LAYOUT)   # bf16 elements
PAIRF8_OFFS, PAIRF8_TOTAL = _offsets(PAIRF8_LAYOUT)   # fp8 elements
COREF8_OFFS, COREF8_TOTAL = _offsets(COREF8_LAYOUT)   # fp8 elements
F32_OFFS, F32_TOTAL = _offsets(F32_LAYOUT)      # f32 elements
FP8 = ml_dtypes.float8_e4m3

def _pad4(n):
    return (n + 3) & ~3

# byte layout of the oct (8-way shared) weight blob, 1/8 uploaded per core
OCT_BYTES = 2 * _OT
OCT_BYTES += (-OCT_BYTES) % 32768    # eighths stay 4096-aligned for 2-D DMA
OCT_PC = OCT_BYTES // 8
# byte layout of the pair blob: [bf16 region][fp8 region], halved for upload
PAIR_BF_BYTES = _pad4(2 * _PT)
PAIR_BYTES = PAIR_BF_BYTES + _pad4(PAIRF8_TOTAL)
PAIR_BYTES += (-PAIR_BYTES) % 8192   # halves stay 4096-aligned for 2-D DMA
PH_BYTES = PAIR_BYTES // 2
# byte layout of the master wire pack:
#   [oct eighth][pair half][core bf16][core fp8][f32]
PAIR_OFF = OCT_PC
CORE_BF_OFF = PAIR_OFF + PH_BYTES
CORE_F8_OFF = CORE_BF_OFF + _pad4(2 * CORE_TOTAL)
F32_OFF = CORE_F8_OFF + _pad4(COREF8_TOTAL)
NBYTES = F32_OFF + 4 * F32_TOTAL

_CACHE = {}


def _host_prep(inputs):
    """Pack inputs into (8, PAIR_HALF) + (8, CORE_TOTAL) bf16 + (8, F32_TOTAL) f32."""
    f32 = np.float32
    W1 = np.asarray(inputs['Wc1'], f32)
    W2 = np.asarray(inputs['Wc2'], f32)
    W3 = np.asarray(inputs['Wc3'], f32)[0, :, 0, 0]
    b1 = np.asarray(inputs['bc1'], f32)
    b2 = np.asarray(inputs['bc2'], f32)
    b3 = float(np.asarray(inputs['bc3'], f32)[0])
    Wp = np.asarray(inputs['Wp'], f32)
    bp = np.asarray(inputs['bp'], f32)
    Wl, Wr = Wp[:, :DN], Wp[:, DN:]
    WN = np.asarray(inputs['WN'], f32)
    bN = np.asarray(inputs['bN'], f32)
    WE = np.asarray(inputs['WE'], f32)
    bE = np.asarray(inputs['bE'], f32)

    A = W1.sum(axis=3)
    Wv = W1.sum(axis=2)
    cU = np.einsum('oidr,i->od', W1, bp)

    sh = {}
    sh['WNT'] = WN.T
    sh['WETb'] = WE.T
    sh['WlT'] = Wl.T
    sh['WlT16'] = (Wl / 16.0).T
    sh['WrT'] = Wr.T
    sh['WrT16'] = (Wr / 16.0).T

    def pack3(M):  # (co, ci, dl) -> [32, 96] of [ci, co] blocks
        out = np.zeros((32, 96), f32)
        for dl in range(3):
            out[:, 32 * dl:32 * dl + 32] = M[:, :, dl].T
        return out

    sh['UW'] = pack3(A)
    sh['A0W'] = pack3(W1[:, :, :, 1:].sum(axis=3))
    sh['A511W'] = pack3(W1[:, :, :, :2].sum(axis=3))

    W1c0 = np.zeros((32, 192), f32)
    W1c511 = np.zeros((32, 192), f32)
    for dl in range(3):
        for t, dr in enumerate((1, 2)):
            W1c0[:, 32 * (2 * dl + t):32 * (2 * dl + t) + 32] = W1[:, :, dl, dr].T
        for t, dr in enumerate((0, 1)):
            W1c511[:, 32 * (2 * dl + t):32 * (2 * dl + t) + 32] = W1[:, :, dl, dr].T
    sh['W1c0'], sh['W1c511'] = W1c0, W1c511

    c0c = np.zeros((1, 96), f32)
    c511c = np.zeros((1, 96), f32)
    for dl in range(3):
        c0c[0, 32 * dl:32 * dl + 32] = np.einsum('oid,i->o', W1[:, :, dl, 1:], bp)
        c511c[0, 32 * dl:32 * dl + 32] = np.einsum('oid,i->o', W1[:, :, dl, :2], bp)
    c0c[0, 32:64] += b1
    c511c[0, 32:64] += b1
    sh['c0const'], sh['c511const'] = c0c, c511c

    VW = np.zeros((32, 384), f32)
    for dr in range(3):
        blk = Wv[:, :, dr].T
        for j in range(4):
            VW[:, 128 * dr + 32 * j:128 * dr + 32 * j + 32] = blk
    sh['VW'] = VW
    vc = cU.sum(axis=1) + b1
    VC = np.tile(vc, 4).reshape(1, 128)
    sh['VC'] = VC

    W2P0 = np.zeros((128, 96), f32)
    W2P1 = np.zeros((128, 96), f32)
    for dr in range(3):
        for j in range(3):
            W2P0[32 * j:32 * j + 32, 32 * dr:32 * dr + 32] = W2[:, :, j, dr].T
        for j in range(1, 4):
            W2P1[32 * j:32 * j + 32, 32 * dr:32 * dr + 32] = W2[:, :, j - 1, dr].T
    sh['W2P0'], sh['W2P1'] = W2P0, W2P1

    W3sel = np.zeros((128, 4), f32)
    for j in range(4):
        W3sel[32 * j:32 * j + 32, j] = W3
    sh['W3selb'] = W3sel
    sh['ONE1'] = np.ones((1, 1), f32)
    sh['ONESR'] = np.ones((1, 512), f32)

    shf = {}
    shf['gnnbias'] = (bN + bE).reshape(DN, 1)
    shf['bc2rep'] = np.tile(b2, 4).reshape(128, 1)
    shf['b3vec'] = np.full((128, 1), b3, f32)

    # big features: convert to bf16 once, per batch, then slice per core
    lig_nf = np.asarray(inputs['ligand_node_features'], f32).astype(BF16)
    lig_ef = np.asarray(inputs['ligand_edge_features'], f32).astype(FP8)
    rec_nf = np.asarray(inputs['receptor_node_features'], f32).astype(BF16)
    rec_ef = np.asarray(inputs['receptor_edge_features'], f32).astype(FP8)
    lig_nfT = [np.ascontiguousarray(lig_nf[b].T) for b in range(B)]      # (128, L)
    lig_efT = [np.ascontiguousarray(lig_ef[b].reshape(L * KNB, DE).T)
               for b in range(B)]                                        # (64, L*K)
    rec_nfT = [np.ascontiguousarray(rec_nf[b].T) for b in range(B)]
    rec_efT = [np.ascontiguousarray(rec_ef[b].reshape(R * KNB, DE).T)
               for b in range(B)]

    master = np.zeros((8, NBYTES), np.uint8)
    packb = np.zeros((8, CORE_TOTAL), BF16)
    packb8 = np.zeros((8, COREF8_TOTAL), FP8)
    packf = np.zeros((8, F32_TOTAL), np.float32)

    octby = np.zeros(OCT_BYTES, np.uint8)
    octbf = octby[:2 * _OT].view(BF16)
    for name in ('WNT', 'WETb', 'WlT', 'WlT16', 'WrT', 'WrT16', 'UW',
                 'A0W', 'A511W', 'W1c0', 'W1c511', 'c0const', 'c511const',
                 'VW', 'VC', 'W2P0', 'W2P1', 'W3selb', 'ONE1', 'ONESR'):
        off, r, c = OCT_OFFS[name]
        a = np.asarray(sh[name])
        assert a.shape == (r, c), (name, a.shape, (r, c))
        octbf[off:off + r * c] = a.astype(BF16, copy=False).ravel()
    for core in range(8):
        master[core, :OCT_PC] = octby[core * OCT_PC:(core + 1) * OCT_PC]

    for b in range(B):
        blobby = np.zeros(PAIR_BYTES, np.uint8)
        blob = blobby[:2 * _PT].view(BF16)
        off, r, c = PAIR_OFFS['recnodeT']
        blob[off:off + r * c] = rec_nfT[b].ravel()
        o8, r8, c8 = PAIRF8_OFFS['recedgeT']
        blobby[PAIR_BF_BYTES + o8:PAIR_BF_BYTES + o8 + r8 * c8] = \
            rec_efT[b].reshape(-1).view(np.uint8)
        master[2 * b, PAIR_OFF:PAIR_OFF + PH_BYTES] = blobby[:PH_BYTES]
        master[2 * b + 1, PAIR_OFF:PAIR_OFF + PH_BYTES] = blobby[PH_BYTES:]

    for core in range(8):
        b, h = core // 2, core % 2
        lo = 256 * h - 2

        def putb(name, arr):
            off, r, c = CORE_OFFS[name]
            a = np.asarray(arr)
            assert a.shape == (r, c), (name, a.shape, (r, c))
            packb[core, off:off + r * c] = a.astype(BF16, copy=False).ravel()

        def putf(name, arr):
            off, r, c = F32_OFFS[name]
            packf[core, off:off + r * c] = np.asarray(arr, np.float32).ravel()

        for name in ('gnnbias', 'bc2rep', 'b3vec'):
            putf(name, shf[name])

        lig_node = np.zeros((128, NLIG), BF16)
        lig_edge = np.zeros((64, PLIG), FP8)
        g0, g1 = max(lo, 0), min(lo + 260, L)
        lig_node[:, g0 - lo:g1 - lo] = lig_nfT[b][:, g0:g1]
        lig_edge[:, (g0 - lo) * KNB:(g1 - lo) * KNB] = \
            lig_efT[b][:, g0 * KNB:g1 * KNB]
        putb('lignodeT', lig_node)
        o8, r8, c8 = COREF8_OFFS['ligedgeT']
        packb8[core, o8:o8 + r8 * c8] = lig_edge.ravel()

        plmask = np.array([1.0 if 0 <= lo + i < L else 0.0 for i in range(260)],
                          np.float32)
        putb('plmask', np.tile(plmask.reshape(1, 260), (32, 1)))
        putb('plmaskrow', plmask.reshape(1, 260))

        flag0 = 1.0 if h == 0 else 0.0
        flag1 = 1.0 if h == 1 else 0.0
        VWfirst = sh['VW'].copy()
        VWqlast = sh['VW'].copy()
        for dr in range(3):
            VWfirst[:, 128 * dr + 32:128 * dr + 64] -= flag0 * W1[:, :, 0, dr].T
            VWqlast[:, 128 * dr + 64:128 * dr + 96] -= flag1 * W1[:, :, 2, dr].T
        putb('VWfirst', VWfirst)
        putb('VWqlast', VWqlast)

        VCfirst, VCqlast = VC.copy(), VC.copy()
        VCfirst[0, 32:64] -= flag0 * cU[:, 0]
        VCqlast[0, 64:96] -= flag1 * cU[:, 2]
        putb('VCfirst', VCfirst)
        putb('VCqlast', VCqlast)

        rmP0 = np.ones((128, 1), f32)
        rmQ63 = np.ones((128, 1), f32)
        for j in range(4):
            if not (0 <= 256 * h + (j - 1) < L):
                rmP0[32 * j:32 * j + 32] = 0.0
            if not (0 <= 256 * h + (253 + j) < L):
                rmQ63[32 * j:32 * j + 32] = 0.0
        putf('rmP0', rmP0)
        putf('rmQ63', rmQ63)
        master[core, CORE_BF_OFF:CORE_BF_OFF + 2 * CORE_TOTAL] = \
            packb[core].view(np.uint8)
        master[core, CORE_F8_OFF:CORE_F8_OFF + COREF8_TOTAL] = \
            packb8[core].view(np.uint8)
        master[core, F32_OFF:F32_OFF + 4 * F32_TOTAL] = \
            packf[core].view(np.uint8)
    return (master,)


def _build_program():
    import concourse.bacc as bacc
    import concourse.mybir as mybir
    from concourse.tile import TileContext

    dt = mybir.dt
    f32, bf16 = dt.float32, dt.bfloat16
    AF = mybir.ActivationFunctionType
    ALU = mybir.AluOpType

    nc = bacc.Bacc("TRN2", target_bir_lowering=False, debug=False, num_devices=8)

    f8 = dt.float8e4
    u8 = dt.uint8
    masterd = nc.dram_tensor("master", [1, NBYTES], u8, kind="ExternalInput")
    out = nc.dram_tensor("out", [8 * 512, 256], u8, kind="ExternalOutput")

    with TileContext(nc) as tc:
        with tc.tile_pool(name="const", bufs=1) as cpool, \
             tc.tile_pool(name="dstage", bufs=1, space="DRAM") as dpool:

            # ---- oct AllGather: weight eighth -> full weight blob ----
            obounce = dpool.tile([1, OCT_PC], u8)
            goct = dpool.tile([1, OCT_BYTES], u8)
            nc.gpsimd.dma_start(
                out=obounce[:].rearrange("o (r c) -> (o r) c", c=4096),
                in_=masterd[0:1, 0:OCT_PC].rearrange(
                    "o (r c) -> (o r) c", c=4096))
            nc.gpsimd.collective_compute(
                "AllGather", mybir.AluOpType.bypass,
                replica_groups=[[0, 1, 2, 3, 4, 5, 6, 7]],
                ins=[obounce[:]],
                outs=[goct[:].rearrange("o (a b) -> (o a) b", a=8)])

            # ---- pair AllGather: half-blob -> full shared blob (bytes) ----
            pbounce = dpool.tile([1, PH_BYTES], u8)
            gpair = dpool.tile([1, PAIR_BYTES], u8)
            nc.gpsimd.dma_start(
                out=pbounce[:].rearrange("o (r c) -> (o r) c", c=4096),
                in_=masterd[0:1, PAIR_OFF:PAIR_OFF + PH_BYTES].rearrange(
                    "o (r c) -> (o r) c", c=4096))
            nc.gpsimd.collective_compute(
                "AllGather", mybir.AluOpType.bypass,
                replica_groups=[[0, 1], [2, 3], [4, 5], [6, 7]],
                ins=[pbounce[:]],
                outs=[gpair[:].rearrange("o (a b) -> (o a) b", a=2)])

            def osrc(name):
                off, r, c = OCT_OFFS[name]
                return goct[0:1, 2 * off:2 * (off + r * c)].bitcast(
                    bf16).rearrange("o (r c) -> (o r) c", c=c)

            def gsrc(name):
                off, r, c = PAIR_OFFS[name]
                return gpair[0:1, 2 * off:2 * (off + r * c)].bitcast(
                    bf16).rearrange("o (r c) -> (o r) c", c=c)

            def bsrc(name):
                off, r, c = CORE_OFFS[name]
                bo = CORE_BF_OFF + 2 * off
                return masterd[0:1, bo:bo + 2 * r * c].bitcast(
                    bf16).rearrange("o (r c) -> (o r) c", c=c)

            def fsrc(name):
                off, r, c = F32_OFFS[name]
                bo = F32_OFF + 4 * off
                return masterd[0:1, bo:bo + 4 * r * c].bitcast(
                    f32).rearrange("o (r c) -> (o r) c", c=c)

            def ctile(name, dtype=bf16, src=None, offs=None):
                off, r, c = offs[name]
                t = cpool.tile([128, c], dtype, tag=f"c_{name}")
                nc.sync.dma_start(out=t[0:r, 0:c], in_=src(name))
                return t

            def ptile(name):
                return ctile(name, src=osrc, offs=OCT_OFFS)

            def btile(name):
                return ctile(name, src=bsrc, offs=CORE_OFFS)

            def ftile(name):
                return ctile(name, dtype=f32, src=fsrc, offs=F32_OFFS)

            WNT_s = ptile("WNT")
            WETb_s = cpool.tile([128, 128], f8, tag="c_WETb")
            nc.gpsimd.dma_start(out=WETb_s[0:64, 0:128], in_=osrc("WETb"))
            gnnbias_s = ftile("gnnbias")
            WlT_s, WlT16_s = ptile("WlT"), ptile("WlT16")
            WrT_s, WrT16_s = ptile("WrT"), ptile("WrT16")
            UW_s = ptile("UW")
            W1c0_s, W1c511_s = ptile("W1c0"), ptile("W1c511")
            c0c_s, c511c_s = ptile("c0const"), ptile("c511const")
            VW_s, VWf_s, VWq_s = ptile("VW"), btile("VWfirst"), btile("VWqlast")
            VC_s, VCf_s, VCq_s = ptile("VC"), btile("VCfirst"), btile("VCqlast")
            W2P0_s, W2P1_s = ptile("W2P0"), ptile("W2P1")
            W3sel_s = ptile("W3selb")
            bc2rep_s, b3vec_s = ftile("bc2rep"), ftile("b3vec")
            ONE1_s, ONESR_s = ptile("ONE1"), ptile("ONESR")
            plmask_s = btile("plmask")
            plmaskrow_s = btile("plmaskrow")
            rmP0_s, rmQ63_s = ftile("rmP0"), ftile("rmQ63")
            nodeT_lig = btile("lignodeT")
            nodeT_rec = ctile("recnodeT", src=gsrc, offs=PAIR_OFFS)

            lo8 = CORE_F8_OFF + COREF8_OFFS["ligedgeT"][0]
            ligedge_flat = masterd[0:1, lo8:lo8 + 64 * PLIG].bitcast(
                f8).rearrange("o (r c) -> (o r) c", c=PLIG)
            ro8 = PAIR_BF_BYTES + PAIRF8_OFFS["recedgeT"][0]
            recedge_flat = gpair[0:1, ro8:ro8 + 64 * PREC].bitcast(
                f8).rearrange("o (r c) -> (o r) c", c=PREC)

            S_lig = cpool.tile([128, NLIG], bf16)
            S_rec = cpool.tile([128, NREC], bf16)
            plzA = cpool.tile([128, 260], bf16)     # rows 0-31 plz, row 32 mask
            przA = cpool.tile([128, 514], bf16)
            U_sb = cpool.tile([128, 260], f32)
            Uc0_sb = cpool.tile([128, 260], f32)
            Uc511_sb = cpool.tile([128, 260], f32)
            A0AUG = cpool.tile([128, 96], bf16)
            A511AUG = cpool.tile([128, 96], bf16)
            V_rep = cpool.tile([128, 512], f32)
            V_first = cpool.tile([128, 512], f32)
            V_qlast = cpool.tile([128, 512], f32)
            uP = cpool.tile([128, 64], f32, tag="uP")
            uQ = cpool.tile([128, 64], f32, tag="uQ")
            uc0P = cpool.tile([128, 64], f32, tag="uc0P")
            uc0Q = cpool.tile([128, 64], f32, tag="uc0Q")
            uc511P = cpool.tile([128, 64], f32, tag="uc511P")
            uc511Q = cpool.tile([128, 64], f32, tag="uc511Q")

            stage = dpool.tile([512, 256], u8)
            gout = dpool.tile([8 * 512, 256], u8)

            # ================= GNN phase =================
            with tc.tile_pool(name="gnn", bufs=4) as gpool, \
                 tc.tile_pool(name="gpsum", bufs=3, space="PSUM") as gpsum, \
                 tc.tile_pool(name="spsum", bufs=1, space="PSUM") as spsum:

                for (eflat, nodeTb_s, S_s, npos) in (
                        (ligedge_flat, nodeT_lig, S_lig, PLIG),
                        (recedge_flat, nodeT_rec, S_rec, PREC)):
                    nch = npos // KNB
                    starts = list(range(0, nch - nch % CH, CH))
                    sizes = [CH] * len(starts)
                    if nch % CH:
                        starts.append(nch - nch % CH)
                        sizes.append(nch % CH)
                    for c0, csz in zip(starts, sizes):
                        w = csz * KNB
                        et = gpool.tile([128, CH * KNB], f8, tag="edge")
                        nc.gpsimd.dma_start(
                            out=et[0:64, 0:w],
                            in_=eflat[:, c0 * KNB:c0 * KNB + w])
                        hz = gpsum.tile([128, CH * KNB], f32, tag="hz")
                        for q0 in range(0, w, 512):
                            qw = min(512, w - q0)
                            nc.tensor.matmul(
                                hz[:, q0:q0 + qw],
                                WETb_s[0:64, :],
                                et[0:64, q0:q0 + qw],
                                start=True, stop=False)
                            rhs = nodeTb_s[:, c0 + q0 // KNB:c0 + (q0 + qw) // KNB]
                            rhs = rhs.unsqueeze(2).broadcast_to(
                                [128, qw // KNB, KNB])
                            nc.tensor.matmul(
                                hz[:, q0:q0 + qw],
                                WNT_s[:], rhs,
                                start=False, stop=True)
                        zt = gpool.tile([128, CH * KNB], bf16, tag="zt")
                        nc.scalar.activation(zt[:, 0:w], hz[:, 0:w], AF.Tanh,
                                             bias=gnnbias_s[:, 0:1])
                        ztr = zt[:, 0:w].rearrange("p (n k) -> p n k", k=KNB)
                        with nc.allow_low_precision(
                                reason="S in bf16 feeds a bf16 matmul; "
                                       "2e-2 output tolerance"):
                            nc.vector.reduce_sum(
                                S_s[:, c0:c0 + csz], ztr,
                                axis=mybir.AxisListType.X)

                # ---- HOPI ----
                pp = spsum.tile([128, 512], f32, tag="sp")
                nc.tensor.matmul(pp[0:32, 0:NLIG], WlT_s[0:128, :], nodeT_lig[:],
                                 start=True, stop=False)
                nc.tensor.matmul(pp[0:32, 0:NLIG], WlT16_s[0:128, :], S_lig[:],
                                 start=False, stop=True)
                nc.vector.tensor_mul(plzA[0:32, :], pp[0:32, 0:260],
                                     plmask_s[0:32, :])
                nc.sync.dma_start(out=plzA[32:33, :], in_=plmaskrow_s[0:1, 0:260])

                pp2 = spsum.tile([128, 512], f32, tag="sp")
                nc.tensor.matmul(pp2[0:32, 0:NREC], WrT_s[0:128, :], nodeT_rec[:],
                                 start=True, stop=False)
                nc.tensor.matmul(pp2[0:32, 0:NREC], WrT16_s[0:128, :], S_rec[:],
                                 start=False, stop=True)
                nc.vector.memset(przA[0:32, 0:1], 0.0)
                nc.vector.memset(przA[0:32, 513:514], 0.0)
                nc.scalar.activation(przA[0:32, 1:513], pp2[0:32, 0:NREC], AF.Copy)

                # ---- U ----
                up = spsum.tile([128, 512], f32, tag="sp")
                for dl in range(3):
                    nc.tensor.matmul(up[0:32, 0:258],
                                     UW_s[0:32, 32 * dl:32 * dl + 32],
                                     plzA[0:32, dl:dl + 258],
                                     start=(dl == 0), stop=(dl == 2))
                nc.scalar.activation(U_sb[0:32, 0:258], up[0:32, 0:258], AF.Copy)

                # ---- c0 / c511 rows ----
                nc.sync.dma_start(out=A0AUG[0:32, :], in_=osrc("A0W"))
                nc.sync.dma_start(out=A511AUG[0:32, :], in_=osrc("A511W"))
                for which, (W1c_s, cc_s, dst) in enumerate(
                        ((W1c0_s, c0c_s, A0AUG), (W1c511_s, c511c_s, A511AUG))):
                    cp = spsum.tile([128, 512], f32, tag="sp")
                    for dl in range(3):
                        for t in range(2):
                            col = (1 + t) if which == 0 else (511 + t)
                            nc.tensor.matmul(
                                cp[0:1, 32 * dl:32 * dl + 32],
                                przA[0:32, col:col + 1],
                                W1c_s[0:32, 32 * (2 * dl + t):32 * (2 * dl + t) + 32],
                                start=(t == 0), stop=False)
                        nc.tensor.matmul(
                            cp[0:1, 32 * dl:32 * dl + 32],
                            ONE1_s[0:1, 0:1],
                            cc_s[0:1, 32 * dl:32 * dl + 32],
                            start=False, stop=True)
                    nc.scalar.activation(dst[32:33, 0:96], cp[0:1, 0:96], AF.Copy)

                # ---- Ucol0 / Ucol511 ----
                for AUG, dstu in ((A0AUG, Uc0_sb), (A511AUG, Uc511_sb)):
                    ucp = spsum.tile([128, 512], f32, tag="sp")
                    for dl in range(3):
                        nc.tensor.matmul(ucp[0:32, 0:258],
                                         AUG[0:33, 32 * dl:32 * dl + 32],
                                         plzA[0:33, dl:dl + 258],
                                         start=(dl == 0), stop=(dl == 2))
                    nc.scalar.activation(dstu[0:32, 0:258], ucp[0:32, 0:258], AF.Copy)

                # ---- V variants ----
                for VWx, VCx, vt in ((VW_s, VC_s, V_rep), (VWf_s, VCf_s, V_first),
                                     (VWq_s, VCq_s, V_qlast)):
                    vp = spsum.tile([128, 512], f32, tag="sp")
                    for dr in range(3):
                        nc.tensor.matmul(vp[:, 0:512],
                                         VWx[0:32, 128 * dr:128 * dr + 128],
                                         przA[0:32, dr:dr + 512],
                                         start=(dr == 0), stop=False)
                    nc.tensor.matmul(vp[:, 0:512], VCx[0:1, 0:128], ONESR_s[0:1, 0:512],
                                     start=False, stop=True)
                    nc.scalar.activation(vt[:], vp[:, 0:512], AF.Copy)

                # ---- u relayouts (i = 4s+j for P, 4s+2+j for Q) ----
                for (src, dstP, dstQ) in ((U_sb, uP, uQ), (Uc0_sb, uc0P, uc0Q),
                                          (Uc511_sb, uc511P, uc511Q)):
                    srcr = src[0:32, 0:260].rearrange("c (s f) -> c s f", f=4)
                    for j in range(4):
                        nc.sync.dma_start(out=dstP[32 * j:32 * j + 32, 0:64],
                                          in_=srcr[:, 0:64, j])
                    for j in range(2):
                        nc.sync.dma_start(out=dstQ[32 * j:32 * j + 32, 0:64],
                                          in_=srcr[:, 0:64, 2 + j])
                    for j in range(2, 4):
                        nc.sync.dma_start(out=dstQ[32 * j:32 * j + 32, 0:64],
                                          in_=srcr[:, 1:65, j - 2])
                for (t, col, rm) in ((uP, 0, rmP0_s), (uc0P, 0, rmP0_s),
                                     (uc511P, 0, rmP0_s), (uQ, 63, rmQ63_s),
                                     (uc0Q, 63, rmQ63_s), (uc511Q, 63, rmQ63_s)):
                    nc.vector.tensor_mul(t[:, col:col + 1], t[:, col:col + 1], rm[:])

            # ================= conv pipeline =================
            with tc.tile_pool(name="x1", bufs=3) as x1pool, \
                 tc.tile_pool(name="x2", bufs=3) as x2pool, \
                 tc.tile_pool(name="osb", bufs=2) as opool, \
                 tc.tile_pool(name="cpsum", bufs=4, space="PSUM") as cpsum, \
                 tc.tile_pool(name="c3ps", bufs=2, space="PSUM") as c3psum:

                for k in range(NSTRIP):
                    x1P = x1pool.tile([128, 8 * 514], bf16, tag="x1P")
                    x1Q = x1pool.tile([128, 8 * 514], bf16, tag="x1Q")
                    for s in range(8):
                        sg = 8 * k + s
                        for (tile_, uu, Vgen, is_edge, rm) in (
                                (x1P, uP, V_first if sg == 0 else V_rep, sg == 0, rmP0_s),
                                (x1Q, uQ, V_qlast if sg == 63 else V_rep, sg == 63, rmQ63_s)):
                            dst = tile_[:, s * 514 + 1:s * 514 + 513]
                            bias_ap = uu[:, sg:sg + 1]
                            if is_edge:
                                nc.scalar.activation(dst, Vgen[:], AF.Relu,
                                                     bias=bias_ap, scale=rm[:])
                            elif s % 3 == 0:
                                nc.scalar.activation(dst, Vgen[:], AF.Relu, bias=bias_ap)
                            else:
                                nc.vector.tensor_scalar(dst, Vgen[:], bias_ap, 0.0,
                                                        ALU.add, ALU.max)
                    for tile_, ucol0, ucol511 in ((x1P, uc0P, uc511P), (x1Q, uc0Q, uc511Q)):
                        tr = tile_[:].rearrange("p (s c) -> p s c", c=514)
                        nc.vector.memset(tr[:, :, 0], 0.0)
                        nc.vector.memset(tr[:, :, 513], 0.0)
                        nc.vector.tensor_scalar(tr[:, :, 1], ucol0[:, 8 * k:8 * k + 8],
                                                0.0, None, ALU.max)
                        nc.vector.tensor_scalar(tr[:, :, 512], ucol511[:, 8 * k:8 * k + 8],
                                                0.0, None, ALU.max)

                    x2 = x2pool.tile([128, 8 * 512], bf16, tag="x2")
                    for s in range(8):
                        c2 = cpsum.tile([128, 512], f32, tag="c2")
                        for dr in range(3):
                            wp0 = W2P0_s[:, 32 * dr:32 * dr + 32]
                            wp1 = W2P1_s[:, 32 * dr:32 * dr + 32]
                            rhsP = x1P[:, s * 514 + dr:s * 514 + dr + 512]
                            rhsQ = x1Q[:, s * 514 + dr:s * 514 + dr + 512]
                            st, sp_ = (dr == 0), (dr == 2)
                            nc.tensor.matmul(c2[0:32, :], wp0, rhsP, start=st, stop=sp_,
                                             tile_position=(0, 0), skip_group_check=True)
                            nc.tensor.matmul(c2[32:64, :], wp1, rhsP, start=st, stop=sp_,
                                             tile_position=(0, 32), skip_group_check=True)
                            nc.tensor.matmul(c2[64:96, :], wp0, rhsQ, start=st, stop=sp_,
                                             tile_position=(0, 64), skip_group_check=True)
                            nc.tensor.matmul(c2[96:128, :], wp1, rhsQ, start=st, stop=sp_,
                                             tile_position=(0, 96), skip_group_check=True)
                        dst2 = x2[:, s * 512:(s + 1) * 512]
                        if s % 3 != 2:
                            nc.scalar.activation(dst2, c2[:], AF.Relu, bias=bc2rep_s[:, 0:1])
                        else:
                            nc.vector.tensor_scalar(dst2, c2[:], bc2rep_s[:, 0:1], 0.0,
                                                    ALU.add, ALU.max)

                    # conv3: logits transposed onto 128 partitions (r-slab on
                    # partitions, strip-row on free); undone host-side.
                    c3p = c3psum.tile([128, 128], f32, tag="c3")
                    for s in range(8):
                        xc = x2[:, s * 512:(s + 1) * 512]
                        for u in range(4):
                            nc.tensor.matmul(
                                c3p[:, 32 * u + 4 * s:32 * u + 4 * s + 4],
                                xc[:, 128 * u:128 * u + 128],
                                W3sel_s[:, 0:4], start=True, stop=True)
                    osg = opool.tile([128, 128], f32, tag="osg")
                    nc.scalar.activation(osg[:], c3p[:], AF.Sigmoid,
                                         bias=b3vec_s[:, 0:1])
                    osb = opool.tile([128, 128], u8, tag="osb")
                    with nc.allow_low_precision(
                            reason="sigmoid output quantized to 1/255 steps; "
                                   "2e-2 output tolerance"):
                        nc.vector.tensor_scalar(osb[:], osg[:], 255.0, None,
                                                ALU.mult)
                    # osb[p, 32u+4s+m] = sigmoid(logit[row=4s+m, r=128u+p])
                    osr = osb[:].rearrange("p (u c) -> p u c", c=32)
                    outr = stage[:].rearrange("(u p) g -> p u g", p=128)
                    nc.sync.dma_start(out=outr[:, :, 32 * k:32 * k + 32],
                                      in_=osr)

            nc.gpsimd.collective_compute(
                "AllGather", mybir.AluOpType.bypass,
                replica_groups=[[0, 1, 2, 3, 4, 5, 6, 7]],
                ins=[stage.opt()], outs=[gout.opt()])
            nc.gpsimd.dma_start(out=out[:], in_=gout[:])

    nc.compile()
    return nc


def _get_runner():
    """Build (once) the cached jitted SPMD callable + on-device zeros maker."""
    if "runner" in _CACHE:
        return _CACHE["runner"]

    import jax
    import jax.numpy as jnp
    import concourse.mybir as mybir
    from concourse.bass2jax import (install_neuronx_cc_hook, _bass_exec_p,
                                    partition_id_tensor)
    from jax.sharding import Mesh, PartitionSpec, NamedSharding
    from jax.experimental.shard_map import shard_map

    nc = _build_program()
    install_neuronx_cc_hook()

    partition_name = nc.partition_id_tensor.name if nc.partition_id_tensor else None
    in_names, out_names, out_avals = [], [], []
    for alloc in nc.m.functions[0].allocations:
        if not isinstance(alloc, mybir.MemoryLocationSet):
            continue
        name = alloc.memorylocations[0].name
        if alloc.kind == "ExternalInput":
            if name != partition_name:
                in_names.append(name)
        elif alloc.kind == "ExternalOutput":
            out_names.append(name)
            out_avals.append(jax.core.ShapedArray(tuple(alloc.tensor_shape),
                                                  mybir.dt.np(alloc.dtype)))
    n_params, n_outs = len(in_names), len(out_avals)
    assert in_names == ["master"] and out_names == ["out"], (in_names, out_names)
    in_names_full = list(in_names) + list(out_names)
    if partition_name is not None:
        in_names_full.append(partition_name)

    def _body(*args):
        operands = list(args)
        if partition_name is not None:
            operands.append(partition_id_tensor())
        return tuple(_bass_exec_p.bind(
            *operands, out_avals=tuple(out_avals), in_names=tuple(in_names_full),
            out_names=tuple(out_names), lowering_input_output_aliases=(),
            sim_require_finite=True, sim_require_nnan=True, nc=nc))

    mesh = Mesh(np.asarray(jax.devices()[:8]), ("core",))
    sharded = jax.jit(
        shard_map(_body, mesh=mesh,
                  in_specs=(PartitionSpec("core"),) * (n_params + n_outs),
                  out_specs=(PartitionSpec("core"),) * n_outs, check_rep=False),
        donate_argnums=tuple(range(n_params, n_params + n_outs)),
        keep_unused=True)

    oshape = tuple(out_avals[0].shape)
    odtype = out_avals[0].dtype
    zmaker = jax.jit(lambda: jnp.zeros((8 * oshape[0],) + oshape[1:], odtype),
                     out_shardings=NamedSharding(mesh, PartitionSpec("core")))

    def run(master):
        """-> gathered output (4096, 256) uint8 from core 0 (one round trip)."""
        z = _CACHE.pop("znext", None)
        if z is None:
            z = zmaker()
        out_arrs = sharded(master, z)
        g = out_arrs[0]
        shard0 = min(g.addressable_shards, key=lambda s: s.index[0].start or 0)
        res = np.asarray(shard0.data)
        _CACHE["znext"] = zmaker()   # ready before the next call
        return res

    _CACHE["runner"] = run
    return run


def kernel(**inputs):
    run = _get_runner()
    packs = _host_prep(inputs)
    gathered = run(*packs)
    full = np.empty((B, L, R), np.float32)
    for core in range(8):
        b, h = core // 2, core % 2
        full[b, 256 * h:256 * h + 256, :] = \
            gathered[512 * core:512 * core + 512].astype(np.float32).T
    full *= (1.0 / 255.0)
    return full
